# revision 1
# baseline (speedup 1.0000x reference)
"""MGDPR (gnn_message_passing) Trainium2 kernel, 8 NeuronCores.

Sharding: nodes row-sharded 4-way within each batch element; cores 0-3 own
batch 0, cores 4-7 own batch 1 (375 nodes each). adj is host-transposed to
[m, n] tiles so the diffusion matmul contracts over the source-node dim on
partitions. All per-node tensors live channel-major ([C, nodes]) on chip so
every matmul streams >=256 free-dim columns (full-rate fp32r). h is
re-gathered across the 4 cores of each batch after layers 0 and 1 via
AllGather. h_prime never depends on node data (zeros init + per-channel
affine), so its contribution folds into a per-layer bias on the host.
"""

import numpy as np

try:
    import concourse.bass as bass
except ImportError:
    import sys

    sys.path.insert(0, "/opt/trn_rl_repo")
    import concourse.bass as bass

import concourse.mybir as mybir
import concourse.tile as tile
from concourse import bacc
from concourse.bass_utils import run_bass_kernel_spmd

B, N, T, DIN, C, R, K, L, H, OUT = 2, 1500, 20, 32, 128, 5, 5, 3, 4, 2
HD = C // H
EPS = 1e-5
NCORES = 8
NS = N // 4          # 375 nodes per core
NSP = NS + 1         # padded to even for fp32r ISA rules
NT = 12              # m tiles (1500 padded to 1536)
MPAD = NT * 128
RG = [[0, 1, 2, 3], [4, 5, 6, 7]]
F32R = mybir.dt.float32r
F32 = mybir.dt.float32
BF16 = mybir.dt.bfloat16

_NC_CACHE = {}


def _build_nc():
    if "nc" in _NC_CACHE:
        return _NC_CACHE["nc"]
    nc = bacc.Bacc(None, target_bir_lowering=False, debug=False, num_devices=NCORES)

    # ---- per-core inputs ----
    adjt = nc.dram_tensor("adjt", [NT, R, 128, NSP], BF16, kind="ExternalInput")
    xt = nc.dram_tensor("xt", [DIN + 1, MPAD], F32R, kind="ExternalInput")
    # ---- replicated consts (host-prelaid in SBUF layout, partition-first) ----
    wp_d = nc.dram_tensor("wp", [C, L * R * C], F32R, kind="ExternalInput")
    qkvo_d = nc.dram_tensor("qkvo", [C, L * 4 * C], F32R, kind="ExternalInput")
    w2at_d = nc.dram_tensor("w2at", [C, L * C], F32R, kind="ExternalInput")
    ow1t_d = nc.dram_tensor("ow1t", [C, C], F32R, kind="ExternalInput")
    ow2t_d = nc.dram_tensor("ow2t", [C, OUT], F32R, kind="ExternalInput")
    embt_d = nc.dram_tensor("embt", [DIN + 1, C], F32R, kind="ExternalInput")
    esum_d = nc.dram_tensor("esum", [C, H], F32R, kind="ExternalInput")
    escore_d = nc.dram_tensor("escore", [C, H], F32R, kind="ExternalInput")
    e01t_d = nc.dram_tensor("e01t", [H, C], F32R, kind="ExternalInput")
    eg_d = nc.dram_tensor("eg", [H, L * C], F32R, kind="ExternalInput")
    ident_d = nc.dram_tensor("ident", [C, C], F32R, kind="ExternalInput")
    cols_d = nc.dram_tensor("cols", [C, 7 * L + 3], F32R, kind="ExternalInput")

    outt = nc.dram_tensor("outt", [OUT, NSP], F32R, kind="ExternalOutput")

    # gather bounce buffers (internal DRAM), one pair per inter-layer gather
    zp_in = nc.dram_tensor("zp_in", [MPAD - N, C], BF16)
    zp_out = nc.dram_tensor("zp_out", [MPAD - N, C], BF16)
    g_in = [nc.dram_tensor(f"g_in_{l}", [NS, C], BF16) for l in range(2)]
    g_out = [nc.dram_tensor(f"g_out_{l}", [MPAD, C], BF16) for l in range(2)]

    with tile.TileContext(nc) as tc:
        with (
            tc.tile_pool(name="persist", bufs=1) as pers,
            tc.tile_pool(name="work", bufs=2) as work,
            tc.tile_pool(name="zwork", bufs=6) as zwork,
            tc.tile_pool(name="small", bufs=2) as small,
            tc.tile_pool(name="pz", bufs=2, space="PSUM") as pz,
            tc.tile_pool(name="pm", bufs=2, space="PSUM") as pm,
            tc.tile_pool(name="pp", bufs=4, space="PSUM") as pp,
        ):
            # ---------- resident tensors ----------
            adjsb = pers.tile([128, NT, R, NSP], BF16, tag="adjsb")
            hnat = pers.tile([128, NT * 128], BF16, tag="hnat")
            xtsb = pers.tile([DIN + 1, MPAD], F32R, tag="xtsb")
            wpsb = pers.tile([C, L * R * C], F32R, tag="wpsb")
            qkvosb = pers.tile([C, L * 4 * C], F32R, tag="qkvosb")
            w2atsb = pers.tile([C, L * C], F32R, tag="w2atsb")
            ow1tsb = pers.tile([C, C], F32R, tag="ow1tsb")
            ow2tsb = pers.tile([C, OUT], F32R, tag="ow2tsb")
            embtsb = pers.tile([DIN + 1, C], F32R, tag="embtsb")
            esumsb = pers.tile([C, H], F32R, tag="esumsb")
            escoresb = pers.tile([C, H], F32R, tag="escoresb")
            e01tsb = pers.tile([H, C], F32R, tag="e01tsb")
            egsb = pers.tile([H, L * C], F32R, tag="egsb")
            identsb = pers.tile([C, C], F32R, tag="identsb")
            colsb = pers.tile([C, 7 * L + 3], F32R, tag="colsb")
            zpadsb = pers.tile([MPAD - N, C], BF16, tag="zpadsb")

            for mt in range(2):
                nc.sync.dma_start(
                    adjsb[:, mt, :, :],
                    adjt[mt].rearrange("r mi j -> mi r j"),
                )
            for dst, src in (
                (xtsb, xt), (embtsb, embt_d),
                (wpsb, wp_d), (qkvosb, qkvo_d), (w2atsb, w2at_d),
                (ow1tsb, ow1t_d), (ow2tsb, ow2t_d),
                (esumsb, esum_d), (escoresb, escore_d), (e01tsb, e01t_d),
                (egsb, eg_d), (identsb, ident_d), (colsb, cols_d),
            ):
                nc.sync.dma_start(dst[:], src[:, :])

            # zero the 36 pad rows of the gather outputs once; the AllReduce
            # of the zero tile doubles as an all-core start barrier so the
            # first real AllGather doesn't absorb the launch skew
            nc.vector.memset(zpadsb[:], 0.0)
            nc.sync.dma_start(zp_in[:, :], zpadsb[:])
            nc.gpsimd.collective_compute(
                "AllReduce", mybir.AluOpType.add,
                replica_groups=[list(range(NCORES))],
                ins=[zp_in[:, :].opt()], outs=[zp_out[:, :].opt()],
            )
            for l in range(2):
                nc.sync.dma_start(g_out[l][N:MPAD, :], zp_out[:, :])

            # adj shard: one DMA per m-tile (~480 KB each), mt-major so layer 0
            # can start consuming early; split across two queues
            for mt in range(2, NT):
                eng = nc.sync if mt % 2 == 0 else nc.gpsimd
                eng.dma_start(
                    adjsb[:, mt, :, :],
                    adjt[mt].rearrange("r mi j -> mi r j"),
                )

            def col(i):
                return colsb[:, i : i + 1]

            wp4 = wpsb.rearrange("p (l r co) -> p l r co", l=L, r=R)
            qk4 = qkvosb.rearrange("p (l i co) -> p l i co", l=L, i=4)
            w2a3 = w2atsb.rearrange("p (l co) -> p l co", l=L)
            eg3 = egsb.rearrange("p (l co) -> p l co", l=L)

            # ---------- h1 = embedding (natural layout, m on partitions) ----------
            for mt in range(NT):
                ep = pp.tile([128, 128], F32, tag="ps")
                nc.tensor.matmul(
                    ep[:], xtsb[:, mt * 128 : (mt + 1) * 128], embtsb[:],
                    start=True, stop=True,
                )
                if mt % 2 == 0:
                    nc.vector.tensor_copy(hnat[:, mt * 128 : (mt + 1) * 128], ep[:])
                else:
                    nc.scalar.copy(hnat[:, mt * 128 : (mt + 1) * 128], ep[:])

            # ---------- layers ----------
            for l in range(L):
                # diffusion + relational merge
                mps = pm.tile([128, NSP], F32, tag="m")
                for r in range(R):
                    zps = pz.tile([128, NSP], F32, tag="z")
                    for mt in range(NT):
                        nc.tensor.matmul(
                            zps[:],
                            hnat[:, mt * 128 : (mt + 1) * 128],
                            adjsb[:, mt, r, :],
                            start=(mt == 0), stop=(mt == NT - 1),
                            skip_group_check=True,
                        )
                    zsb = zwork.tile([128, NSP], F32R, tag="zsb")
                    if r % 2 == 0:
                        nc.scalar.copy(zsb[:], zps[:])
                    else:
                        nc.vector.tensor_copy(zsb[:], zps[:])
                    nc.tensor.matmul(
                        mps[:], wp4[:, l, r, :], zsb[:],
                        start=(r == 0), stop=(r == R - 1),
                        skip_group_check=True,
                    )
                hdT = work.tile([128, NSP], F32R, tag="hdT")
                nc.scalar.activation(
                    hdT[:], mps[:], mybir.ActivationFunctionType.Relu,
                    bias=col(7 * l + 6), scale=1.0,
                )

                # retention projections
                qps = pp.tile([128, NSP], F32, tag="ps")
                nc.tensor.matmul(qps[:], qk4[:, l, 0, :], hdT[:], start=True, stop=True)
                qsb = work.tile([128, NSP], F32R, tag="qsb")
                nc.scalar.activation(
                    qsb[:], qps[:], mybir.ActivationFunctionType.Identity,
                    bias=col(7 * l + 0),
                )
                kps = pp.tile([128, NSP], F32, tag="ps")
                nc.tensor.matmul(kps[:], qk4[:, l, 1, :], hdT[:], start=True, stop=True)
                ksb = work.tile([128, NSP], F32R, tag="ksb")
                nc.scalar.activation(
                    ksb[:], kps[:], mybir.ActivationFunctionType.Identity,
                    bias=col(7 * l + 1),
                )
                vps = pp.tile([128, NSP], F32, tag="ps")
                nc.tensor.matmul(vps[:], qk4[:, l, 2, :], hdT[:], start=True, stop=True)
                vsb = work.tile([128, NSP], F32R, tag="vsb")
                nc.scalar.activation(
                    vsb[:], vps[:], mybir.ActivationFunctionType.Identity,
                    bias=col(7 * l + 2),
                )

                qk = work.tile([128, NSP], F32R, tag="qk")
                nc.vector.tensor_mul(qk[:], qsb[:], ksb[:])
                sps = pp.tile([H, NSP], F32, tag="ps")
                nc.tensor.matmul(sps[:], escoresb[:], qk[:], start=True, stop=True)
                ssb = small.tile([H, NSP], F32R, tag="ssb")
                nc.scalar.copy(ssb[:], sps[:])
                sbps = pp.tile([128, NSP], F32, tag="ps")
                nc.tensor.matmul(sbps[:], e01tsb[:], ssb[:], start=True, stop=True)
                osb = work.tile([128, NSP], F32R, tag="osb")
                nc.vector.tensor_mul(osb[:], vsb[:], sbps[:])

                o2ps = pp.tile([128, NSP], F32, tag="ps")
                nc.tensor.matmul(o2ps[:], qk4[:, l, 3, :], osb[:], start=True, stop=True)
                o2sb = work.tile([128, NSP], F32R, tag="o2sb")
                nc.scalar.activation(
                    o2sb[:], o2ps[:], mybir.ActivationFunctionType.Identity,
                    bias=col(7 * l + 3),
                )

                # group norm over (head, node): stats via E-matmuls
                sq = work.tile([128, NSP], F32R, tag="sq")
                nc.vector.tensor_mul(sq[:], o2sb[:], o2sb[:])
                mups = pp.tile([H, NSP], F32, tag="ps")
                nc.tensor.matmul(mups[:], esumsb[:], o2sb[:], start=True, stop=True)
                msps = pp.tile([H, NSP], F32, tag="ps")
                nc.tensor.matmul(msps[:], esumsb[:], sq[:], start=True, stop=True)
                musb = small.tile([H, NSP], F32R, tag="musb")
                nc.scalar.copy(musb[:], mups[:])
                mu2 = small.tile([H, NSP], F32R, tag="mu2")
                nc.vector.tensor_mul(mu2[:], musb[:], musb[:])
                varsb = small.tile([H, NSP], F32R, tag="varsb")
                nc.vector.tensor_sub(varsb[:], msps[:], mu2[:])
                stdsb = small.tile([H, NSP], F32R, tag="stdsb")
                nc.scalar.activation(
                    stdsb[:], varsb[:], mybir.ActivationFunctionType.Sqrt,
                    bias=colsb[0:H, 7 * L + 2 : 7 * L + 3],
                )
                rstd = small.tile([H, NSP], F32R, tag="rstd")
                with nc.allow_low_precision(reason="f32r is f32 storage"):
                    nc.vector.reciprocal(rstd[:], stdsb[:])
                nmr = small.tile([H, NSP], F32R, tag="nmr")
                nc.vector.scalar_tensor_tensor(
                    nmr[:], musb[:], -1.0, rstd[:],
                    mybir.AluOpType.mult, mybir.AluOpType.mult,
                )
                scaleps = pp.tile([128, NSP], F32, tag="ps")
                nc.tensor.matmul(scaleps[:], eg3[:, l, :], rstd[:], start=True, stop=True)
                shiftps = pp.tile([128, NSP], F32, tag="ps")
                nc.tensor.matmul(shiftps[:], eg3[:, l, :], nmr[:], start=True, stop=True)
                t1 = work.tile([128, NSP], F32R, tag="t1")
                nc.vector.tensor_mul(t1[:], o2sb[:], scaleps[:])
                hrT = work.tile([128, NSP], F32R, tag="hrT")
                nc.vector.scalar_tensor_tensor(
                    hrT[:], t1[:], col(7 * l + 4), shiftps[:],
                    mybir.AluOpType.add, mybir.AluOpType.add,
                )

                # h update (h_prime contribution folded into bias on host)
                h2ps = pp.tile([128, NSP], F32, tag="ps")
                nc.tensor.matmul(h2ps[:], w2a3[:, l, :], hrT[:], start=True, stop=True)
                hnT = work.tile([128, NSP], F32R, tag="hnT")
                nc.scalar.activation(
                    hnT[:], h2ps[:], mybir.ActivationFunctionType.Relu,
                    bias=col(7 * l + 5),
                )

                if l < 2:
                    # transpose shard to natural layout, AllGather, rebuild hnat
                    for c0, cw in ((0, 128), (128, 128), (256, NSP - 256)):
                        trp = pp.tile([cw, 128], F32R, tag="ps")
                        nc.tensor.transpose(trp[:], hnT[:, c0 : c0 + cw], identsb[:])
                        gsb = work.tile([cw, 128], BF16, tag="gsb")
                        nc.scalar.copy(gsb[:], trp[:])
                        rows = min(cw, NS - c0)
                        nc.sync.dma_start(g_in[l][c0 : c0 + rows, :], gsb[:rows, :])
                    nc.gpsimd.collective_compute(
                        "AllGather", mybir.AluOpType.bypass,
                        replica_groups=RG,
                        ins=[g_in[l][:, :].opt()],
                        outs=[g_out[l][:N, :].opt()],
                    )
                    nc.sync.dma_start(
                        hnat.rearrange("p (mt c) -> p mt c", mt=NT),
                        g_out[l][:, :].rearrange("(mt mi) c -> mi mt c", mi=128),
                    )
                else:
                    # final head
                    hmps = pp.tile([128, NSP], F32, tag="ps")
                    nc.tensor.matmul(hmps[:], ow1tsb[:], hnT[:], start=True, stop=True)
                    hmsb = work.tile([128, NSP], F32R, tag="hmsb")
                    nc.scalar.activation(
                        hmsb[:], hmps[:], mybir.ActivationFunctionType.Relu,
                        bias=col(7 * L),
                    )
                    oops = pp.tile([OUT, NSP], F32, tag="ps")
                    nc.tensor.matmul(oops[:], ow2tsb[:], hmsb[:], start=True, stop=True)
                    oosb = small.tile([OUT, NSP], F32R, tag="oosb")
                    nc.scalar.activation(
                        oosb[:], oops[:], mybir.ActivationFunctionType.Identity,
                        bias=colsb[0:OUT, 7 * L + 1 : 7 * L + 2],
                    )
                    nc.sync.dma_start(outt[:, :], oosb[:])

    nc.finalize()
    _NC_CACHE["nc"] = nc
    return nc


def _prep(inputs):
    f32 = np.float32

    def g(name):
        return np.asarray(inputs[name], f32)

    x, adj = g("x"), g("adj_list")
    alpha, transition = g("alpha"), g("transition")
    conv_w, conv_b = g("conv_w"), g("conv_b")
    w1, b1, eb1 = g("w1"), g("b1"), g("eb1")
    w2, b2, eb2 = g("w2"), g("b2"), g("eb2")

    a = alpha - alpha.max(-1, keepdims=True)
    e = np.exp(a)
    srow = (e / e.sum(-1, keepdims=True)).sum(-1)          # [L,R]
    Wm = transition.mean(axis=2)                            # [L,R,C,C]
    Wp = (conv_w * srow)[:, :, None, None] * np.swapaxes(Wm, -1, -2)

    hp = np.zeros((C,), f32)
    b2eff = np.zeros((L, C), f32)
    for l in range(L):
        b2eff[l] = b2[l] + eb2[l] + w2[l][:, C:] @ hp
        hp = np.maximum(hp @ w1[l].T + b1[l] + eb1[l], 0.0).astype(f32)

    qkvo = np.stack(
        [np.swapaxes(g(w), -1, -2) for w in ("qw", "kw", "vw", "ow")], axis=1
    )  # [L,4,C,C] in lhsT layout

    hid = np.arange(C) // HD
    ind = (hid[:, None] == np.arange(H)[None, :]).astype(f32)  # [C,H]

    cols = np.zeros((C, 7 * L + 3), f32)
    for l in range(L):
        cols[:, 7 * l + 0] = g("qb")[l]
        cols[:, 7 * l + 1] = g("kb")[l]
        cols[:, 7 * l + 2] = g("vb")[l]
        cols[:, 7 * l + 3] = g("ob")[l]
        cols[:, 7 * l + 4] = g("gn_b")[l]
        cols[:, 7 * l + 5] = b2eff[l]
        cols[:, 7 * l + 6] = conv_b[l]
    cols[:, 7 * L] = g("out_b1")
    cols[:OUT, 7 * L + 1] = g("out_b2")
    cols[:, 7 * L + 2] = EPS

    consts = {
        "wp": np.ascontiguousarray(Wp.transpose(2, 0, 1, 3)).reshape(C, L * R * C),
        "qkvo": np.ascontiguousarray(qkvo.transpose(2, 0, 1, 3)).reshape(C, L * 4 * C),
        "w2at": np.ascontiguousarray(
            np.swapaxes(w2[:, :, :C], -1, -2).transpose(1, 0, 2)
        ).reshape(C, L * C),
        "ow1t": np.ascontiguousarray(g("out_w1").T),
        "ow2t": np.ascontiguousarray(g("out_w2").T),
        "embt": np.concatenate([g("emb_w").T, g("emb_b")[None, :]], axis=0),
        "esum": ind / HD,
        "escore": ind,
        "e01t": np.ascontiguousarray(ind.T),
        "eg": np.ascontiguousarray(
            (ind.T[None] * g("gn_g")[:, None, :]).transpose(1, 0, 2)
        ).reshape(H, L * C),
        "ident": np.eye(C, dtype=f32),
        "cols": cols,
    }

    xlast = x[:, :, -1, :]                                   # [B,N,DIN]
    in_maps = []
    for k in range(NCORES):
        b, s = k // 4, k % 4
        asub = adj[b][:, s * NS : (s + 1) * NS, :]           # [R,NS,N]
        ap = np.zeros((R, NSP, MPAD), f32)
        ap[:, :NS, :N] = asub
        a3 = (
            ap.transpose(2, 0, 1)                            # [MPAD,R,NSP]
            .reshape(NT, 128, R, NSP)
            .transpose(0, 2, 1, 3)                           # [NT,R,128,NSP]
        )
        xt = np.zeros((DIN + 1, MPAD), f32)
        xt[:DIN, :N] = xlast[b].T
        xt[DIN, :] = 1.0
        import ml_dtypes
        in_maps.append(
            dict(consts, adjt=np.ascontiguousarray(a3).astype(ml_dtypes.bfloat16), xt=xt)
        )
    return in_maps


def kernel(**inputs):
    nc = _build_nc()
    in_maps = _prep(inputs)
    res = run_bass_kernel_spmd(nc, in_maps, core_ids=list(range(NCORES)))
    out = np.zeros((B, N, OUT), np.float32)
    for k in range(NCORES):
        b, s = k // 4, k % 4
        out[b, s * NS : (s + 1) * NS, :] = res.results[k]["outt"][:, :NS].T
    return out



# revision 4
# speedup vs baseline: 1.0551x; 1.0551x over previous
"""MGDPR (gnn_message_passing) Trainium2 kernel, 8 NeuronCores.

Sharding: nodes row-sharded 4-way within each batch element; cores 0-3 own
batch 0, cores 4-7 own batch 1 (375 nodes each, padded to 384). The source
(m) axis uses a padded block order: rank k's nodes sit at m = k*384 + j, so
the AllGather output maps 1:1 onto SBUF m-tiles. adj is host-cast to fp8e4
and pre-laid in the exact SBUF layout ([mi][mt, r, j]) so the load is a few
large fully-contiguous DMAs. All per-node tensors are channel-major on chip
([C, nodes]); h is kept bf16. The embedding is fused into layer-0 diffusion
mt-major so compute rides the adjacency DMA stream. Between layers, h is
gathered c-major over the 4 cores of each batch (bf16, one AllGather), and
m-tiles are rebuilt with 12 DMA-transposes (xbar). GroupNorm's affine
(gn_g/gn_b) is folded into w2/bias on the host; h_prime's contribution
(zeros init + affine) folds into a per-layer bias.
"""

import numpy as np

try:
    import concourse.bass as bass
except ImportError:
    import sys

    sys.path.insert(0, "/opt/trn_rl_repo")
    import concourse.bass as bass

import ml_dtypes
import concourse.mybir as mybir
import concourse.tile as tile
from concourse import bacc
from concourse.bass_utils import run_bass_kernel_spmd

B, N, T, DIN, C, R, K, L, H, OUT = 2, 1500, 20, 32, 128, 5, 5, 3, 4, 2
HD = C // H
EPS = 1e-5
NCORES = 8
NS = 375            # real nodes per core
NSP = 384           # padded nodes per core (3 * 128)
NT = 12             # m tiles: 4 ranks * 3 tiles
MPAD = NT * 128     # 1536 = 4 * NSP
RG = [[0, 1, 2, 3], [4, 5, 6, 7]]
F32 = mybir.dt.float32
F32R = mybir.dt.float32r
BF16 = mybir.dt.bfloat16
FP8 = mybir.dt.float8e4
NCOL = 6 * L + 3    # bias columns

_NC_CACHE = {}


def _build_nc():
    if "nc" in _NC_CACHE:
        return _NC_CACHE["nc"]
    nc = bacc.Bacc(None, target_bir_lowering=False, debug=False, num_devices=NCORES)

    adjt = nc.dram_tensor("adjt", [128, NT * R * NSP], FP8, kind="ExternalInput")
    xt = nc.dram_tensor("xt", [DIN + 1, MPAD], BF16, kind="ExternalInput")
    embt_d = nc.dram_tensor("embt", [DIN + 1, C], BF16, kind="ExternalInput")
    wp_d = nc.dram_tensor("wp", [C, L * R * C], BF16, kind="ExternalInput")
    qkvo_d = nc.dram_tensor("qkvo", [C, L * 4 * C], BF16, kind="ExternalInput")
    w2at_d = nc.dram_tensor("w2at", [C, L * C], BF16, kind="ExternalInput")
    ow1t_d = nc.dram_tensor("ow1t", [C, C], BF16, kind="ExternalInput")
    ow2t_d = nc.dram_tensor("ow2t", [C, OUT], BF16, kind="ExternalInput")
    esum_d = nc.dram_tensor("esum", [C, H], BF16, kind="ExternalInput")
    escore_d = nc.dram_tensor("escore", [C, H], BF16, kind="ExternalInput")
    e01t_d = nc.dram_tensor("e01t", [H, C], BF16, kind="ExternalInput")
    cols_d = nc.dram_tensor("cols", [C, NCOL], F32R, kind="ExternalInput")

    outt = nc.dram_tensor("outt", [OUT, NSP], F32R, kind="ExternalOutput")

    g_in = [nc.dram_tensor(f"g_in_{l}", [C, NSP], BF16) for l in range(2)]
    g_out = [nc.dram_tensor(f"g_out_{l}", [4, C, NSP], BF16) for l in range(2)]

    with tile.TileContext(nc) as tc:
        with (
            tc.tile_pool(name="persist", bufs=1) as pers,
            tc.tile_pool(name="work", bufs=2) as work,
            tc.tile_pool(name="zwork", bufs=2) as zwork,
            tc.tile_pool(name="small", bufs=2) as small,
            tc.tile_pool(name="pz", bufs=5, space="PSUM") as pz,
            tc.tile_pool(name="pp", bufs=2, space="PSUM") as pp,
            tc.tile_pool(name="pm", bufs=1, space="PSUM") as pm,
        ):
            # ---------- resident tensors ----------
            adjsb = pers.tile([128, NT, R, NSP], FP8, tag="adjsb")
            hnat = pers.tile([128, NT * 128], BF16, tag="hnat")
            xtsb = pers.tile([DIN + 1, MPAD], BF16, tag="xtsb")
            embtsb = pers.tile([DIN + 1, C], BF16, tag="embtsb")
            wpsb = pers.tile([C, L * R * C], BF16, tag="wpsb")
            qkvosb = pers.tile([C, L * 4 * C], BF16, tag="qkvosb")
            w2atsb = pers.tile([C, L * C], BF16, tag="w2atsb")
            ow1tsb = pers.tile([C, C], BF16, tag="ow1tsb")
            ow2tsb = pers.tile([C, OUT], BF16, tag="ow2tsb")
            esumsb = pers.tile([C, H], BF16, tag="esumsb")
            escoresb = pers.tile([C, H], BF16, tag="escoresb")
            e01tsb = pers.tile([H, C], BF16, tag="e01tsb")
            colsb = pers.tile([C, NCOL], F32R, tag="colsb")

            # small consts first (scalar queue), then xt, then adj stream
            for dst, src in (
                (embtsb, embt_d), (esumsb, esum_d), (escoresb, escore_d),
                (e01tsb, e01t_d), (colsb, cols_d), (qkvosb, qkvo_d),
                (wpsb, wp_d), (w2atsb, w2at_d), (ow1tsb, ow1t_d),
                (ow2tsb, ow2t_d),
            ):
                nc.scalar.dma_start(dst[:], src[:, :])
            nc.sync.dma_start(xtsb[:], xt[:, :])
            adjflat = adjsb.rearrange("p a b c -> p (a b c)")
            CH = 2 * R * NSP
            for ch in range(NT // 2):
                eng = nc.sync if ch % 2 == 0 else nc.gpsimd
                eng.dma_start(
                    adjflat[:, ch * CH : (ch + 1) * CH],
                    adjt[:, ch * CH : (ch + 1) * CH],
                )

            def col(i):
                return colsb[:, i : i + 1]

            wp4 = wpsb.rearrange("p (l r co) -> p l r co", l=L, r=R)
            qk4 = qkvosb.rearrange("p (l i co) -> p l i co", l=L, i=4)
            w2a3 = w2atsb.rearrange("p (l co) -> p l co", l=L)

            def embed(mt):
                ep = pp.tile([128, 128], F32, tag="pp")
                nc.tensor.matmul(
                    ep[:], xtsb[:, mt * 128 : (mt + 1) * 128], embtsb[:],
                    start=True, stop=True, skip_group_check=True,
                )
                if mt % 2 == 0:
                    nc.vector.tensor_copy(hnat[:, mt * 128 : (mt + 1) * 128], ep[:])
                else:
                    nc.scalar.copy(hnat[:, mt * 128 : (mt + 1) * 128], ep[:])

            for l in range(L):
                # ---- diffusion, mt-major with 5 accumulators ----
                if l == 0:
                    embed(0)
                    embed(1)
                zps = [
                    pz.tile([128, NSP], F32, tag="z", name=f"zp_{l}_{r}")
                    for r in range(R)
                ]
                for mt in range(NT):
                    if l == 0 and mt + 2 < NT:
                        embed(mt + 2)
                    for r in range(R):
                        nc.tensor.matmul(
                            zps[r][:],
                            hnat[:, mt * 128 : (mt + 1) * 128],
                            adjsb[:, mt, r, :],
                            start=(mt == 0), stop=(mt == NT - 1),
                            skip_group_check=True,
                        )
                mps = pm.tile([128, NSP], F32, tag="m")
                for r in range(R):
                    zsb = zwork.tile([128, NSP], BF16, tag="zsb")
                    if r % 2 == 0:
                        nc.scalar.copy(zsb[:], zps[r][:])
                    else:
                        nc.vector.tensor_copy(zsb[:], zps[r][:])
                    nc.tensor.matmul(
                        mps[:], wp4[:, l, r, :], zsb[:],
                        start=(r == 0), stop=(r == R - 1),
                        skip_group_check=True,
                    )
                hdT = work.tile([128, NSP], BF16, tag="hdT")
                nc.scalar.activation(
                    hdT[:], mps[:], mybir.ActivationFunctionType.Relu,
                    bias=col(6 * l + 5), scale=1.0,
                )

                # ---- retention (S=1), channel-major ----
                qps = pp.tile([128, NSP], F32, tag="pp")
                nc.tensor.matmul(qps[:], qk4[:, l, 0, :], hdT[:], start=True, stop=True)
                qsb = work.tile([128, NSP], BF16, tag="qsb")
                nc.scalar.activation(
                    qsb[:], qps[:], mybir.ActivationFunctionType.Identity,
                    bias=col(6 * l + 0),
                )
                kps = pp.tile([128, NSP], F32, tag="pp")
                nc.tensor.matmul(kps[:], qk4[:, l, 1, :], hdT[:], start=True, stop=True)
                ksb = work.tile([128, NSP], BF16, tag="ksb")
                nc.scalar.activation(
                    ksb[:], kps[:], mybir.ActivationFunctionType.Identity,
                    bias=col(6 * l + 1),
                )
                qk = work.tile([128, NSP], BF16, tag="qk")
                nc.vector.tensor_mul(qk[:], qsb[:], ksb[:])
                vps = pp.tile([128, NSP], F32, tag="pp")
                nc.tensor.matmul(vps[:], qk4[:, l, 2, :], hdT[:], start=True, stop=True)
                vsb = work.tile([128, NSP], BF16, tag="vsb")
                nc.scalar.activation(
                    vsb[:], vps[:], mybir.ActivationFunctionType.Identity,
                    bias=col(6 * l + 2),
                )
                sps = pp.tile([H, NSP], F32, tag="pp")
                nc.tensor.matmul(sps[:], escoresb[:], qk[:], start=True, stop=True)
                ssb = small.tile([H, NSP], BF16, tag="ssb")
                nc.vector.tensor_copy(ssb[:], sps[:])
                sbps = pp.tile([128, NSP], F32, tag="pp")
                nc.tensor.matmul(sbps[:], e01tsb[:], ssb[:], start=True, stop=True)
                osb = work.tile([128, NSP], BF16, tag="osb")
                nc.vector.tensor_mul(osb[:], vsb[:], sbps[:])

                o2ps = pp.tile([128, NSP], F32, tag="pp")
                nc.tensor.matmul(o2ps[:], qk4[:, l, 3, :], osb[:], start=True, stop=True)
                o2sb = work.tile([128, NSP], BF16, tag="o2sb")
                nc.scalar.activation(
                    o2sb[:], o2ps[:], mybir.ActivationFunctionType.Identity,
                    bias=col(6 * l + 3),
                )

                # group norm stats over (head, node) via E-matmuls
                sq = work.tile([128, NSP], BF16, tag="sq")
                nc.vector.tensor_mul(sq[:], o2sb[:], o2sb[:])
                mups = pp.tile([H, NSP], F32, tag="pp")
                nc.tensor.matmul(mups[:], esumsb[:], o2sb[:], start=True, stop=True)
                msps = pp.tile([H, NSP], F32, tag="pp")
                nc.tensor.matmul(msps[:], esumsb[:], sq[:], start=True, stop=True)
                musb = small.tile([H, NSP], BF16, tag="musb")
                nc.scalar.copy(musb[:], mups[:])
                mu2 = small.tile([H, NSP], BF16, tag="mu2")
                nc.vector.tensor_mul(mu2[:], musb[:], musb[:])
                varsb = small.tile([H, NSP], BF16, tag="varsb")
                nc.vector.tensor_sub(varsb[:], msps[:], mu2[:])
                stdsb = small.tile([H, NSP], BF16, tag="stdsb")
                nc.scalar.activation(
                    stdsb[:], varsb[:], mybir.ActivationFunctionType.Sqrt,
                    bias=colsb[0:H, 6 * L + 2 : 6 * L + 3],
                )
                rstd = small.tile([H, NSP], BF16, tag="rstd")
                with nc.allow_low_precision(reason="groupnorm rstd in bf16"):
                    nc.vector.reciprocal(rstd[:], stdsb[:])
                nmr = small.tile([H, NSP], BF16, tag="nmr")
                nc.vector.scalar_tensor_tensor(
                    nmr[:], musb[:], -1.0, rstd[:],
                    mybir.AluOpType.mult, mybir.AluOpType.mult,
                )
                scaleps = pp.tile([128, NSP], F32, tag="pp")
                nc.tensor.matmul(scaleps[:], e01tsb[:], rstd[:], start=True, stop=True)
                shiftps = pp.tile([128, NSP], F32, tag="pp")
                nc.tensor.matmul(shiftps[:], e01tsb[:], nmr[:], start=True, stop=True)
                t1 = work.tile([128, NSP], BF16, tag="t1")
                nc.vector.tensor_mul(t1[:], o2sb[:], scaleps[:])
                hrT = work.tile([128, NSP], BF16, tag="hrT")
                nc.vector.tensor_add(hrT[:], t1[:], shiftps[:])

                # h update (gn affine + h_prime folded on host)
                h2ps = pp.tile([128, NSP], F32, tag="pp")
                nc.tensor.matmul(h2ps[:], w2a3[:, l, :], hrT[:], start=True, stop=True)
                hnT = work.tile([128, NSP], BF16, tag="hnT")
                nc.scalar.activation(
                    hnT[:], h2ps[:], mybir.ActivationFunctionType.Relu,
                    bias=col(6 * l + 4),
                )

                if l < 2:
                    # c-major gather + xbar-transpose rebuild of m-tiles
                    nc.sync.dma_start(g_in[l][:, :], hnT[:])
                    nc.gpsimd.collective_compute(
                        "AllGather", mybir.AluOpType.bypass,
                        replica_groups=RG,
                        ins=[g_in[l][:, :].opt()],
                        outs=[g_out[l][:, :, :].opt()],
                    )
                    for t in range(NT):
                        kk, j = t // 3, t % 3
                        eng = nc.sync if t % 2 == 0 else nc.scalar
                        eng.dma_start(
                            hnat[:, t * 128 : (t + 1) * 128],
                            g_out[l][kk, :, j * 128 : (j + 1) * 128],
                            transpose=True,
                        )
                else:
                    hmps = pp.tile([128, NSP], F32, tag="pp")
                    nc.tensor.matmul(hmps[:], ow1tsb[:], hnT[:], start=True, stop=True)
                    hmsb = work.tile([128, NSP], BF16, tag="hmsb")
                    nc.scalar.activation(
                        hmsb[:], hmps[:], mybir.ActivationFunctionType.Relu,
                        bias=col(6 * L),
                    )
                    oops = pp.tile([OUT, NSP], F32, tag="pp")
                    nc.tensor.matmul(oops[:], ow2tsb[:], hmsb[:], start=True, stop=True)
                    oosb = small.tile([OUT, NSP], F32R, tag="oosb")
                    nc.scalar.activation(
                        oosb[:], oops[:], mybir.ActivationFunctionType.Identity,
                        bias=colsb[0:OUT, 6 * L + 1 : 6 * L + 2],
                    )
                    nc.sync.dma_start(outt[:, :], oosb[:])

    nc.finalize()
    _NC_CACHE["nc"] = nc
    return nc


def _prep(inputs):
    f32 = np.float32

    def g(name):
        return np.asarray(inputs[name], f32)

    x, adj = g("x"), g("adj_list")
    alpha, transition = g("alpha"), g("transition")
    conv_w, conv_b = g("conv_w"), g("conv_b")
    w1, b1, eb1 = g("w1"), g("b1"), g("eb1")
    w2, b2, eb2 = g("w2"), g("b2"), g("eb2")
    gn_g, gn_b = g("gn_g"), g("gn_b")

    a = alpha - alpha.max(-1, keepdims=True)
    e = np.exp(a)
    srow = (e / e.sum(-1, keepdims=True)).sum(-1)          # [L,R]
    Wm = transition.mean(axis=2)                            # [L,R,C,C]
    Wp = (conv_w * srow)[:, :, None, None] * np.swapaxes(Wm, -1, -2)

    # h_prime path and groupnorm affine folded into the layer bias
    hp = np.zeros((C,), f32)
    b2eff = np.zeros((L, C), f32)
    for l in range(L):
        b2eff[l] = b2[l] + eb2[l] + w2[l][:, C:] @ hp + w2[l][:, :C] @ gn_b[l]
        hp = np.maximum(hp @ w1[l].T + b1[l] + eb1[l], 0.0).astype(f32)

    qkvo = np.stack(
        [np.swapaxes(g(w), -1, -2) for w in ("qw", "kw", "vw", "ow")], axis=1
    )  # [L,4,C,C] lhsT layout

    # w2a with gn_g folded: lhsT[c, o] = w2[l, o, c] * gn_g[l, c]
    w2at = np.swapaxes(w2[:, :, :C], -1, -2) * gn_g[:, :, None]  # [L,C,C]

    hid = np.arange(C) // HD
    ind = (hid[:, None] == np.arange(H)[None, :]).astype(f32)  # [C,H]

    cols = np.zeros((C, NCOL), f32)
    for l in range(L):
        cols[:, 6 * l + 0] = g("qb")[l]
        cols[:, 6 * l + 1] = g("kb")[l]
        cols[:, 6 * l + 2] = g("vb")[l]
        cols[:, 6 * l + 3] = g("ob")[l]
        cols[:, 6 * l + 4] = b2eff[l]
        cols[:, 6 * l + 5] = conv_b[l]
    cols[:, 6 * L] = g("out_b1")
    cols[:OUT, 6 * L + 1] = g("out_b2")
    cols[:, 6 * L + 2] = EPS

    bf = ml_dtypes.bfloat16
    consts = {
        "embt": np.concatenate([g("emb_w").T, g("emb_b")[None, :]], axis=0).astype(bf),
        "wp": np.ascontiguousarray(Wp.transpose(2, 0, 1, 3)).reshape(C, L * R * C).astype(bf),
        "qkvo": np.ascontiguousarray(qkvo.transpose(2, 0, 1, 3)).reshape(C, L * 4 * C).astype(bf),
        "w2at": np.ascontiguousarray(w2at.transpose(1, 0, 2)).reshape(C, L * C).astype(bf),
        "ow1t": np.ascontiguousarray(g("out_w1").T).astype(bf),
        "ow2t": np.ascontiguousarray(g("out_w2").T).astype(bf),
        "esum": (ind / HD).astype(bf),
        "escore": ind.astype(bf),
        "e01t": np.ascontiguousarray(ind.T).astype(bf),
        "cols": cols,
    }

    xlast = x[:, :, -1, :]                                   # [B,N,DIN]
    fp8 = ml_dtypes.float8_e4m3
    in_maps = []
    for core in range(NCORES):
        b, s = core // 4, core % 4
        n0 = s * NS
        # adj block layout: A[m_pad, r, j] with m_pad = rank*384 + jj
        Ab = np.zeros((R, NSP, 4, NSP), f32)                 # [r, j, rank, jj]
        for s2 in range(4):
            Ab[:, :NS, s2, :NS] = adj[b][:, n0 : n0 + NS, s2 * NS : (s2 + 1) * NS]
        a3 = (
            Ab.transpose(2, 3, 0, 1)                          # [rank, jj, r, j]
            .reshape(MPAD, R, NSP)
            .reshape(NT, 128, R, NSP)
            .transpose(1, 0, 2, 3)                            # [mi, mt, r, j]
        )
        xtc = np.zeros((DIN + 1, MPAD), f32)
        xtc[DIN, :] = 1.0
        xv = xlast[b].T                                       # [DIN, N]
        for s2 in range(4):
            xtc[:DIN, s2 * NSP : s2 * NSP + NS] = xv[:, s2 * NS : (s2 + 1) * NS]
        in_maps.append(
            dict(
                consts,
                adjt=np.ascontiguousarray(a3).reshape(128, NT * R * NSP).astype(fp8),
                xt=xtc.astype(bf),
            )
        )
    return in_maps


def kernel(**inputs):
    nc = _build_nc()
    in_maps = _prep(inputs)
    res = run_bass_kernel_spmd(nc, in_maps, core_ids=list(range(NCORES)))
    out = np.zeros((B, N, OUT), np.float32)
    for core in range(NCORES):
        b, s = core // 4, core % 4
        out[b, s * NS : (s + 1) * NS, :] = np.asarray(
            res.results[core]["outt"], np.float32
        )[:, :NS].T
    return out


# revision 8
# speedup vs baseline: 1.0868x; 1.0301x over previous
"""MGDPR (gnn_message_passing) Trainium2 kernel, 8 NeuronCores.

Sharding: nodes row-sharded 4-way within each batch element; cores 0-3 own
batch 0, cores 4-7 own batch 1 (375 nodes each, padded to 384). The source
(m) axis uses a padded block order: rank k's nodes sit at m = k*384 + j, so
the AllGather output maps 1:1 onto SBUF m-tiles. adj is host-cast to fp8e4
and pre-laid in the exact SBUF layout ([mi][mt, r, j]) so the load is a few
large fully-contiguous DMAs. All per-node tensors are channel-major on chip
([C, nodes]); h is kept bf16. The embedding is fused into layer-0 diffusion
mt-major so compute rides the adjacency DMA stream. Between layers, h is
gathered c-major over the 4 cores of each batch (bf16, one AllGather), and
m-tiles are rebuilt with 12 DMA-transposes (xbar). GroupNorm's affine
(gn_g/gn_b) is folded into w2/bias on the host; h_prime's contribution
(zeros init + affine) folds into a per-layer bias.
"""

import numpy as np

try:
    import concourse.bass as bass
except ImportError:
    import sys

    sys.path.insert(0, "/opt/trn_rl_repo")
    import concourse.bass as bass

import ml_dtypes
import concourse.mybir as mybir
import concourse.tile as tile
from concourse import bacc
from concourse.bass_utils import run_bass_kernel_spmd

B, N, T, DIN, C, R, K, L, H, OUT = 2, 1500, 20, 32, 128, 5, 5, 3, 4, 2
HD = C // H
EPS = 1e-5
NCORES = 8
NS = 375            # real nodes per core
NSP = 384           # padded nodes per core (3 * 128)
NT = 12             # m tiles: 4 ranks * 3 tiles
MPAD = NT * 128     # 1536 = 4 * NSP
RG = [[0, 1, 2, 3], [4, 5, 6, 7]]
F32 = mybir.dt.float32
F32R = mybir.dt.float32r
BF16 = mybir.dt.bfloat16
FP8 = mybir.dt.float8e4
NCOL = 6 * L + 3    # bias columns

_NC_CACHE = {}


def _build_nc():
    if "nc" in _NC_CACHE:
        return _NC_CACHE["nc"]
    nc = bacc.Bacc(None, target_bir_lowering=False, debug=False, num_devices=NCORES)

    adjt = nc.dram_tensor("adjt", [128, NT * R * NSP], FP8, kind="ExternalInput")
    xt = nc.dram_tensor("xt", [DIN + 1, MPAD], BF16, kind="ExternalInput")
    embt_d = nc.dram_tensor("embt", [DIN + 1, C], BF16, kind="ExternalInput")
    wp_d = nc.dram_tensor("wp", [C, L * R * C], BF16, kind="ExternalInput")
    qkvo_d = nc.dram_tensor("qkvo", [C, L * 4 * C], BF16, kind="ExternalInput")
    w2at_d = nc.dram_tensor("w2at", [C, L * C], BF16, kind="ExternalInput")
    ow1t_d = nc.dram_tensor("ow1t", [C, C], BF16, kind="ExternalInput")
    ow2t_d = nc.dram_tensor("ow2t", [C, OUT], BF16, kind="ExternalInput")
    indb_d = nc.dram_tensor("indb", [C, C], BF16, kind="ExternalInput")
    inds_d = nc.dram_tensor("inds", [C, C], BF16, kind="ExternalInput")
    cols_d = nc.dram_tensor("cols", [C, NCOL], F32, kind="ExternalInput")

    outt = nc.dram_tensor("outt", [OUT, NSP], F32R, kind="ExternalOutput")

    g_in = [nc.dram_tensor(f"g_in_{l}", [C, NSP], BF16) for l in range(2)]
    g_out = [nc.dram_tensor(f"g_out_{l}", [4, C, NSP], BF16) for l in range(2)]
    warm_in = nc.dram_tensor("warm_in", [1, 64], BF16)
    warm_out = nc.dram_tensor("warm_out", [4, 64], BF16)

    with tile.TileContext(nc) as tc:
        with (
            tc.tile_pool(name="persist", bufs=1) as pers,
            tc.tile_pool(name="work", bufs=2) as work,
            tc.tile_pool(name="zwork", bufs=2) as zwork,
            tc.tile_pool(name="small", bufs=2) as small,
            tc.tile_pool(name="pz", bufs=5, space="PSUM") as pz,
            tc.tile_pool(name="pp", bufs=2, space="PSUM") as pp,
            tc.tile_pool(name="pm", bufs=1, space="PSUM") as pm,
        ):
            # ---------- resident tensors ----------
            adjsb = pers.tile([128, NT, R, NSP], FP8, tag="adjsb")
            hnat = pers.tile([128, NT * 128], BF16, tag="hnat")
            xtsb = pers.tile([DIN + 1, MPAD], BF16, tag="xtsb")
            embtsb = pers.tile([DIN + 1, C], BF16, tag="embtsb")
            wpsb = pers.tile([C, L * R * C], BF16, tag="wpsb")
            qkvosb = pers.tile([C, L * 4 * C], BF16, tag="qkvosb")
            w2atsb = pers.tile([C, L * C], BF16, tag="w2atsb")
            ow1tsb = pers.tile([C, C], BF16, tag="ow1tsb")
            ow2tsb = pers.tile([C, OUT], BF16, tag="ow2tsb")
            indbsb = pers.tile([C, C], BF16, tag="indbsb")
            indssb = pers.tile([C, C], BF16, tag="indssb")
            colsb = pers.tile([C, NCOL], F32, tag="colsb")

            # fire a tiny AllGather immediately: absorbs the cc-stream
            # arming + all-core rendezvous so the first real gather is cheap
            nc.gpsimd.collective_compute(
                "AllGather", mybir.AluOpType.bypass,
                replica_groups=RG,
                ins=[warm_in[:, :].opt()],
                outs=[warm_out[:, :].opt()],
            )
            # small consts first (scalar queue), then xt, then adj stream
            for dst, src in (
                (embtsb, embt_d), (indbsb, indb_d), (indssb, inds_d),
                (colsb, cols_d), (qkvosb, qkvo_d),
                (wpsb, wp_d), (w2atsb, w2at_d), (ow1tsb, ow1t_d),
                (ow2tsb, ow2t_d),
            ):
                nc.scalar.dma_start(dst[:], src[:, :])
            nc.sync.dma_start(xtsb[:], xt[:, :])
            adjflat = adjsb.rearrange("p a b c -> p (a b c)")
            CH = 2 * R * NSP
            for ch in range(NT // 2):
                eng = nc.sync if ch % 2 == 0 else nc.gpsimd
                eng.dma_start(
                    adjflat[:, ch * CH : (ch + 1) * CH],
                    adjt[:, ch * CH : (ch + 1) * CH],
                )

            def col(i):
                return colsb[:, i : i + 1]

            wp4 = wpsb.rearrange("p (l r co) -> p l r co", l=L, r=R)
            qk4 = qkvosb.rearrange("p (l i co) -> p l i co", l=L, i=4)
            w2a3 = w2atsb.rearrange("p (l co) -> p l co", l=L)

            def embed(mt):
                ep = pp.tile([128, 128], F32, tag="pp")
                nc.tensor.matmul(
                    ep[:], xtsb[:, mt * 128 : (mt + 1) * 128], embtsb[:],
                    start=True, stop=True, skip_group_check=True,
                )
                if mt % 2 == 0:
                    nc.vector.tensor_copy(hnat[:, mt * 128 : (mt + 1) * 128], ep[:])
                else:
                    nc.scalar.copy(hnat[:, mt * 128 : (mt + 1) * 128], ep[:])

            for l in range(L):
                # ---- diffusion, mt-major with 5 accumulators ----
                if l == 0:
                    embed(0)
                    embed(1)
                zps = [
                    pz.tile([128, NSP], F32, tag="z", name=f"zp_{l}_{r}")
                    for r in range(R)
                ]
                for mt in range(NT):
                    if l == 0 and mt + 2 < NT:
                        embed(mt + 2)
                    for r in range(R):
                        nc.tensor.matmul(
                            zps[r][:],
                            hnat[:, mt * 128 : (mt + 1) * 128],
                            adjsb[:, mt, r, :],
                            start=(mt == 0), stop=(mt == NT - 1),
                            skip_group_check=True,
                        )
                mps = pm.tile([128, NSP], F32, tag="m")
                for r in range(R):
                    zsb = zwork.tile([128, NSP], BF16, tag="zsb")
                    if r % 2 == 0:
                        nc.scalar.copy(zsb[:], zps[r][:])
                    else:
                        nc.vector.tensor_copy(zsb[:], zps[r][:])
                    nc.tensor.matmul(
                        mps[:], wp4[:, l, r, :], zsb[:],
                        start=(r == 0), stop=(r == R - 1),
                        skip_group_check=True,
                    )
                hdT = work.tile([128, NSP], BF16, tag="hdT")
                nc.scalar.activation(
                    hdT[:], mps[:], mybir.ActivationFunctionType.Relu,
                    bias=col(6 * l + 5), scale=1.0,
                )

                # ---- retention (S=1), channel-major, block matmuls ----
                qps = pp.tile([128, NSP], F32, tag="pp")
                nc.tensor.matmul(qps[:], qk4[:, l, 0, :], hdT[:], start=True, stop=True)
                kps = pp.tile([128, NSP], F32, tag="pp")
                nc.tensor.matmul(kps[:], qk4[:, l, 1, :], hdT[:], start=True, stop=True)
                qsb = work.tile([128, NSP], BF16, tag="qsb")
                nc.scalar.activation(
                    qsb[:], qps[:], mybir.ActivationFunctionType.Identity,
                    bias=col(6 * l + 0),
                )
                ksb = work.tile([128, NSP], BF16, tag="ksb")
                nc.vector.tensor_scalar_add(ksb[:], kps[:], col(6 * l + 1))
                qk = work.tile([128, NSP], BF16, tag="qk")
                nc.vector.tensor_mul(qk[:], qsb[:], ksb[:])
                sbps = pp.tile([128, NSP], F32, tag="pp")
                nc.tensor.matmul(sbps[:], indbsb[:], qk[:], start=True, stop=True)
                vps = pp.tile([128, NSP], F32, tag="pp")
                nc.tensor.matmul(vps[:], qk4[:, l, 2, :], hdT[:], start=True, stop=True)
                vsb = work.tile([128, NSP], BF16, tag="vsb")
                nc.scalar.activation(
                    vsb[:], vps[:], mybir.ActivationFunctionType.Identity,
                    bias=col(6 * l + 2),
                )
                osb = work.tile([128, NSP], BF16, tag="osb")
                nc.vector.tensor_mul(osb[:], vsb[:], sbps[:])

                o2ps = pp.tile([128, NSP], F32, tag="pp")
                nc.tensor.matmul(o2ps[:], qk4[:, l, 3, :], osb[:], start=True, stop=True)
                o2sb = work.tile([128, NSP], BF16, tag="o2sb")
                nc.vector.tensor_copy(o2sb[:], o2ps[:])
                # centered o2 (+ centered ob folded on host): ctr = o2 + obc - mu
                mups = pp.tile([128, NSP], F32, tag="pp")
                nc.tensor.matmul(mups[:], indssb[:], o2sb[:], start=True, stop=True)
                ctr = work.tile([128, NSP], BF16, tag="ctr")
                nc.vector.scalar_tensor_tensor(
                    ctr[:], o2sb[:], col(6 * l + 3), mups[:],
                    mybir.AluOpType.add, mybir.AluOpType.subtract,
                )
                d2 = work.tile([128, NSP], BF16, tag="d2")
                nc.scalar.activation(
                    d2[:], ctr[:], mybir.ActivationFunctionType.Square,
                )
                vrps = pp.tile([128, NSP], F32, tag="pp")
                nc.tensor.matmul(vrps[:], indssb[:], d2[:], start=True, stop=True)
                stdf = work.tile([128, NSP], BF16, tag="stdf")
                nc.scalar.activation(
                    stdf[:], vrps[:], mybir.ActivationFunctionType.Sqrt,
                    bias=col(6 * L + 2),
                )
                rstdf = work.tile([128, NSP], BF16, tag="rstdf")
                with nc.allow_low_precision(reason="groupnorm rstd in bf16"):
                    nc.vector.reciprocal(rstdf[:], stdf[:])
                hrT = work.tile([128, NSP], BF16, tag="hrT")
                nc.vector.tensor_mul(hrT[:], ctr[:], rstdf[:])

                # h update (gn affine + h_prime folded on host)
                h2ps = pp.tile([128, NSP], F32, tag="pp")
                nc.tensor.matmul(h2ps[:], w2a3[:, l, :], hrT[:], start=True, stop=True)
                hnT = work.tile([128, NSP], BF16, tag="hnT")
                nc.scalar.activation(
                    hnT[:], h2ps[:], mybir.ActivationFunctionType.Relu,
                    bias=col(6 * l + 4),
                )

                if l < 2:
                    # c-major gather + xbar-transpose rebuild of m-tiles
                    nc.sync.dma_start(g_in[l][:, :], hnT[:])
                    nc.gpsimd.collective_compute(
                        "AllGather", mybir.AluOpType.bypass,
                        replica_groups=RG,
                        ins=[g_in[l][:, :].opt()],
                        outs=[g_out[l][:, :, :].opt()],
                    )
                    for t in range(NT):
                        kk, j = t // 3, t % 3
                        eng = nc.sync if t % 2 == 0 else nc.scalar
                        eng.dma_start(
                            hnat[:, t * 128 : (t + 1) * 128],
                            g_out[l][kk, :, j * 128 : (j + 1) * 128],
                            transpose=True,
                        )
                else:
                    hmps = pp.tile([128, NSP], F32, tag="pp")
                    nc.tensor.matmul(hmps[:], ow1tsb[:], hnT[:], start=True, stop=True)
                    hmsb = work.tile([128, NSP], BF16, tag="hmsb")
                    nc.scalar.activation(
                        hmsb[:], hmps[:], mybir.ActivationFunctionType.Relu,
                        bias=col(6 * L),
                    )
                    oops = pp.tile([OUT, NSP], F32, tag="pp")
                    nc.tensor.matmul(oops[:], ow2tsb[:], hmsb[:], start=True, stop=True)
                    oosb = small.tile([OUT, NSP], F32R, tag="oosb")
                    nc.scalar.activation(
                        oosb[:], oops[:], mybir.ActivationFunctionType.Identity,
                        bias=colsb[0:OUT, 6 * L + 1 : 6 * L + 2],
                    )
                    nc.sync.dma_start(outt[:, :], oosb[:])

    nc.finalize()
    _NC_CACHE["nc"] = nc
    return nc


def _prep(inputs):
    f32 = np.float32

    def g(name):
        return np.asarray(inputs[name], f32)

    x, adj = g("x"), g("adj_list")
    alpha, transition = g("alpha"), g("transition")
    conv_w, conv_b = g("conv_w"), g("conv_b")
    w1, b1, eb1 = g("w1"), g("b1"), g("eb1")
    w2, b2, eb2 = g("w2"), g("b2"), g("eb2")
    gn_g, gn_b = g("gn_g"), g("gn_b")

    a = alpha - alpha.max(-1, keepdims=True)
    e = np.exp(a)
    srow = (e / e.sum(-1, keepdims=True)).sum(-1)          # [L,R]
    Wm = transition.mean(axis=2)                            # [L,R,C,C]
    Wp = (conv_w * srow)[:, :, None, None] * np.swapaxes(Wm, -1, -2)

    # h_prime path and groupnorm affine folded into the layer bias
    hp = np.zeros((C,), f32)
    b2eff = np.zeros((L, C), f32)
    for l in range(L):
        b2eff[l] = b2[l] + eb2[l] + w2[l][:, C:] @ hp + w2[l][:, :C] @ gn_b[l]
        hp = np.maximum(hp @ w1[l].T + b1[l] + eb1[l], 0.0).astype(f32)

    qkvo = np.stack(
        [np.swapaxes(g(w), -1, -2) for w in ("qw", "kw", "vw", "ow")], axis=1
    )  # [L,4,C,C] lhsT layout

    # w2a with gn_g folded: lhsT[c, o] = w2[l, o, c] * gn_g[l, c]
    w2at = np.swapaxes(w2[:, :, :C], -1, -2) * gn_g[:, :, None]  # [L,C,C]

    hid = np.arange(C) // HD
    same = (hid[:, None] == hid[None, :]).astype(f32)          # [C,C] same-head

    # centered ob: the part of ob surviving groupnorm mean subtraction
    ob = g("ob")
    obc = ob - ob @ (same / HD).T                               # [L,C]

    cols = np.zeros((C, NCOL), f32)
    for l in range(L):
        cols[:, 6 * l + 0] = g("qb")[l]
        cols[:, 6 * l + 1] = g("kb")[l]
        cols[:, 6 * l + 2] = g("vb")[l]
        cols[:, 6 * l + 3] = obc[l]
        cols[:, 6 * l + 4] = b2eff[l]
        cols[:, 6 * l + 5] = conv_b[l]
    cols[:, 6 * L] = g("out_b1")
    cols[:OUT, 6 * L + 1] = g("out_b2")
    cols[:, 6 * L + 2] = EPS

    bf = ml_dtypes.bfloat16
    consts = {
        "embt": np.concatenate([g("emb_w").T, g("emb_b")[None, :]], axis=0).astype(bf),
        "wp": np.ascontiguousarray(Wp.transpose(2, 0, 1, 3)).reshape(C, L * R * C).astype(bf),
        "qkvo": np.ascontiguousarray(qkvo.transpose(2, 0, 1, 3)).reshape(C, L * 4 * C).astype(bf),
        "w2at": np.ascontiguousarray(w2at.transpose(1, 0, 2)).reshape(C, L * C).astype(bf),
        "ow1t": np.ascontiguousarray(g("out_w1").T).astype(bf),
        "ow2t": np.ascontiguousarray(g("out_w2").T).astype(bf),
        "indb": same.astype(bf),
        "inds": (same / HD).astype(bf),
        "cols": cols,
    }

    xlast = x[:, :, -1, :]                                   # [B,N,DIN]
    fp8 = ml_dtypes.float8_e4m3
    in_maps = []
    for core in range(NCORES):
        b, s = core // 4, core % 4
        n0 = s * NS
        # adj block layout: A[m_pad, r, j] with m_pad = rank*384 + jj
        Ab = np.zeros((R, NSP, 4, NSP), f32)                 # [r, j, rank, jj]
        for s2 in range(4):
            Ab[:, :NS, s2, :NS] = adj[b][:, n0 : n0 + NS, s2 * NS : (s2 + 1) * NS]
        a3 = (
            Ab.transpose(2, 3, 0, 1)                          # [rank, jj, r, j]
            .reshape(MPAD, R, NSP)
            .reshape(NT, 128, R, NSP)
            .transpose(1, 0, 2, 3)                            # [mi, mt, r, j]
        )
        xtc = np.zeros((DIN + 1, MPAD), f32)
        xtc[DIN, :] = 1.0
        xv = xlast[b].T                                       # [DIN, N]
        for s2 in range(4):
            xtc[:DIN, s2 * NSP : s2 * NSP + NS] = xv[:, s2 * NS : (s2 + 1) * NS]
        in_maps.append(
            dict(
                consts,
                adjt=np.ascontiguousarray(a3).reshape(128, NT * R * NSP).astype(fp8),
                xt=xtc.astype(bf),
            )
        )
    return in_maps


def kernel(**inputs):
    nc = _build_nc()
    in_maps = _prep(inputs)
    res = run_bass_kernel_spmd(nc, in_maps, core_ids=list(range(NCORES)))
    out = np.zeros((B, N, OUT), np.float32)
    for core in range(NCORES):
        b, s = core // 4, core % 4
        out[b, s * NS : (s + 1) * NS, :] = np.asarray(
            res.results[core]["outt"], np.float32
        )[:, :NS].T
    return out


# revision 9
# speedup vs baseline: 1.1693x; 1.0759x over previous
"""MGDPR (gnn_message_passing) Trainium2 kernel, 8 NeuronCores.

Sharding: nodes row-sharded 4-way within each batch element; cores 0-3 own
batch 0, cores 4-7 own batch 1 (375 nodes each, padded to 384). The source
(m) axis uses a padded block order: rank k's nodes sit at m = k*384 + j, so
the AllGather output maps 1:1 onto SBUF m-tiles. adj is host-cast to fp8e4
and pre-laid in the exact SBUF layout ([mi][mt, r, j]) so the load is a few
large fully-contiguous DMAs. All per-node tensors are channel-major on chip
([C, nodes]); h is kept bf16. The embedding is fused into layer-0 diffusion
mt-major so compute rides the adjacency DMA stream. Between layers, h is
gathered c-major over the 4 cores of each batch (bf16, one AllGather), and
m-tiles are rebuilt with 12 DMA-transposes (xbar). GroupNorm's affine
(gn_g/gn_b) is folded into w2/bias on the host; h_prime's contribution
(zeros init + affine) folds into a per-layer bias.
"""

import numpy as np

try:
    import concourse.bass as bass
except ImportError:
    import sys

    sys.path.insert(0, "/opt/trn_rl_repo")
    import concourse.bass as bass

import ml_dtypes
import concourse.mybir as mybir
import concourse.tile as tile
from concourse import bacc
from concourse.bass_utils import run_bass_kernel_spmd

B, N, T, DIN, C, R, K, L, H, OUT = 2, 1500, 20, 32, 128, 5, 5, 3, 4, 2
HD = C // H
EPS = 1e-5
NCORES = 8
NS = 375            # real nodes per core
NSP = 384           # padded nodes per core (3 * 128)
NT = 12             # m tiles: 4 ranks * 3 tiles
MPAD = NT * 128     # 1536 = 4 * NSP
RG = [[0, 1, 2, 3], [4, 5, 6, 7]]
F32 = mybir.dt.float32
F32R = mybir.dt.float32r
BF16 = mybir.dt.bfloat16
FP8 = mybir.dt.float8e4
NCOL = 6 * L + 3    # bias columns

_NC_CACHE = {}


def _build_nc():
    if "nc" in _NC_CACHE:
        return _NC_CACHE["nc"]
    nc = bacc.Bacc(None, target_bir_lowering=False, debug=False, num_devices=NCORES)

    adjt = nc.dram_tensor("adjt", [128, NT * R * NSP], FP8, kind="ExternalInput")
    xt = nc.dram_tensor("xt", [DIN + 1, MPAD], BF16, kind="ExternalInput")
    embt_d = nc.dram_tensor("embt", [DIN + 1, C], BF16, kind="ExternalInput")
    wp_d = nc.dram_tensor("wp", [C, L * R * C], BF16, kind="ExternalInput")
    qkvo_d = nc.dram_tensor("qkvo", [C, L * 4 * C], BF16, kind="ExternalInput")
    w2at_d = nc.dram_tensor("w2at", [C, L * C], BF16, kind="ExternalInput")
    ow1t_d = nc.dram_tensor("ow1t", [C, C], BF16, kind="ExternalInput")
    ow2t_d = nc.dram_tensor("ow2t", [C, OUT], BF16, kind="ExternalInput")
    indb_d = nc.dram_tensor("indb", [C, C], BF16, kind="ExternalInput")
    inds_d = nc.dram_tensor("inds", [C, C], BF16, kind="ExternalInput")
    cols_d = nc.dram_tensor("cols", [C, NCOL], F32, kind="ExternalInput")

    outt = nc.dram_tensor("outt", [OUT, NSP], F32R, kind="ExternalOutput")

    g_in = [nc.dram_tensor(f"g_in_{l}", [C, NSP], BF16) for l in range(2)]
    g_out = [nc.dram_tensor(f"g_out_{l}", [4, C, NSP], BF16) for l in range(2)]

    with tile.TileContext(nc) as tc:
        with (
            tc.tile_pool(name="persist", bufs=1) as pers,
            tc.tile_pool(name="work", bufs=2) as work,
            tc.tile_pool(name="zwork", bufs=2) as zwork,
            tc.tile_pool(name="small", bufs=2) as small,
            tc.tile_pool(name="pz", bufs=5, space="PSUM") as pz,
            tc.tile_pool(name="pp", bufs=2, space="PSUM") as pp,
            tc.tile_pool(name="pm", bufs=1, space="PSUM") as pm,
        ):
            # ---------- resident tensors ----------
            adjsb = pers.tile([128, NT, R, NSP], FP8, tag="adjsb")
            hnat = pers.tile([128, NT * 128], BF16, tag="hnat")
            hnf8 = pers.tile([128, NT * 128], FP8, tag="hnf8")
            xtsb = pers.tile([DIN + 1, MPAD], BF16, tag="xtsb")
            embtsb = pers.tile([DIN + 1, C], BF16, tag="embtsb")
            wpsb = pers.tile([C, L * R * C], BF16, tag="wpsb")
            qkvosb = pers.tile([C, L * 4 * C], BF16, tag="qkvosb")
            w2atsb = pers.tile([C, L * C], BF16, tag="w2atsb")
            ow1tsb = pers.tile([C, C], BF16, tag="ow1tsb")
            ow2tsb = pers.tile([C, OUT], BF16, tag="ow2tsb")
            indbsb = pers.tile([C, C], BF16, tag="indbsb")
            indssb = pers.tile([C, C], BF16, tag="indssb")
            colsb = pers.tile([C, NCOL], F32, tag="colsb")

            # small consts first (scalar queue), then xt, then adj stream
            for dst, src in (
                (embtsb, embt_d), (indbsb, indb_d), (indssb, inds_d),
                (colsb, cols_d), (qkvosb, qkvo_d),
                (wpsb, wp_d), (w2atsb, w2at_d), (ow1tsb, ow1t_d),
                (ow2tsb, ow2t_d),
            ):
                nc.scalar.dma_start(dst[:], src[:, :])
            nc.sync.dma_start(xtsb[:], xt[:, :])
            adjflat = adjsb.rearrange("p a b c -> p (a b c)")
            CH = 2 * R * NSP
            for ch in range(NT // 2):
                eng = nc.sync if ch % 2 == 0 else nc.gpsimd
                eng.dma_start(
                    adjflat[:, ch * CH : (ch + 1) * CH],
                    adjt[:, ch * CH : (ch + 1) * CH],
                )

            def col(i):
                return colsb[:, i : i + 1]

            wp4 = wpsb.rearrange("p (l r co) -> p l r co", l=L, r=R)
            qk4 = qkvosb.rearrange("p (l i co) -> p l i co", l=L, i=4)
            w2a3 = w2atsb.rearrange("p (l co) -> p l co", l=L)

            def embed(mt):
                ep = pp.tile([128, 128], F32, tag="pp")
                nc.tensor.matmul(
                    ep[:], xtsb[:, mt * 128 : (mt + 1) * 128], embtsb[:],
                    start=True, stop=True, skip_group_check=True,
                )
                with nc.allow_low_precision(reason="fp8 h for diffusion"):
                    if mt % 2 == 0:
                        nc.vector.tensor_copy(
                            hnf8[:, mt * 128 : (mt + 1) * 128], ep[:]
                        )
                    else:
                        nc.scalar.copy(hnf8[:, mt * 128 : (mt + 1) * 128], ep[:])

            for l in range(L):
                # ---- diffusion, mt-major with 5 accumulators ----
                if l == 0:
                    embed(0)
                    embed(1)
                zps = [
                    pz.tile([128, NSP], F32, tag="z", name=f"zp_{l}_{r}")
                    for r in range(R)
                ]
                h3 = hnf8.rearrange("p (mt c) -> p mt c", mt=NT)
                for m2 in range(NT // 2):
                    if l == 0 and 2 * m2 + 3 < NT:
                        embed(2 * m2 + 2)
                        embed(2 * m2 + 3)
                    for r in range(R):
                        nc.tensor.matmul(
                            zps[r][:],
                            h3[:, 2 * m2 : 2 * m2 + 2, :],
                            adjsb[:, 2 * m2 : 2 * m2 + 2, r, :],
                            start=(m2 == 0), stop=(m2 == NT // 2 - 1),
                            perf_mode=mybir.MatmulPerfMode.DoubleRow,
                            skip_group_check=True,
                        )
                mps = pm.tile([128, NSP], F32, tag="m")
                for r in range(R):
                    zsb = zwork.tile([128, NSP], BF16, tag="zsb")
                    if r % 2 == 0:
                        nc.scalar.copy(zsb[:], zps[r][:])
                    else:
                        nc.vector.tensor_copy(zsb[:], zps[r][:])
                    nc.tensor.matmul(
                        mps[:], wp4[:, l, r, :], zsb[:],
                        start=(r == 0), stop=(r == R - 1),
                        skip_group_check=True,
                    )
                hdT = work.tile([128, NSP], BF16, tag="hdT")
                nc.scalar.activation(
                    hdT[:], mps[:], mybir.ActivationFunctionType.Relu,
                    bias=col(6 * l + 5), scale=1.0,
                )

                # ---- retention (S=1), channel-major, block matmuls ----
                qps = pp.tile([128, NSP], F32, tag="pp")
                nc.tensor.matmul(qps[:], qk4[:, l, 0, :], hdT[:], start=True, stop=True)
                kps = pp.tile([128, NSP], F32, tag="pp")
                nc.tensor.matmul(kps[:], qk4[:, l, 1, :], hdT[:], start=True, stop=True)
                qsb = work.tile([128, NSP], BF16, tag="qsb")
                nc.scalar.activation(
                    qsb[:], qps[:], mybir.ActivationFunctionType.Identity,
                    bias=col(6 * l + 0),
                )
                ksb = work.tile([128, NSP], BF16, tag="ksb")
                nc.vector.tensor_scalar_add(ksb[:], kps[:], col(6 * l + 1))
                qk = work.tile([128, NSP], BF16, tag="qk")
                nc.vector.tensor_mul(qk[:], qsb[:], ksb[:])
                sbps = pp.tile([128, NSP], F32, tag="pp")
                nc.tensor.matmul(sbps[:], indbsb[:], qk[:], start=True, stop=True)
                vps = pp.tile([128, NSP], F32, tag="pp")
                nc.tensor.matmul(vps[:], qk4[:, l, 2, :], hdT[:], start=True, stop=True)
                vsb = work.tile([128, NSP], BF16, tag="vsb")
                nc.scalar.activation(
                    vsb[:], vps[:], mybir.ActivationFunctionType.Identity,
                    bias=col(6 * l + 2),
                )
                osb = work.tile([128, NSP], BF16, tag="osb")
                nc.vector.tensor_mul(osb[:], vsb[:], sbps[:])

                o2ps = pp.tile([128, NSP], F32, tag="pp")
                nc.tensor.matmul(o2ps[:], qk4[:, l, 3, :], osb[:], start=True, stop=True)
                o2sb = work.tile([128, NSP], BF16, tag="o2sb")
                nc.vector.tensor_copy(o2sb[:], o2ps[:])
                # centered o2 (+ centered ob folded on host): ctr = o2 + obc - mu
                mups = pp.tile([128, NSP], F32, tag="pp")
                nc.tensor.matmul(mups[:], indssb[:], o2sb[:], start=True, stop=True)
                ctr = work.tile([128, NSP], BF16, tag="ctr")
                nc.vector.scalar_tensor_tensor(
                    ctr[:], o2sb[:], col(6 * l + 3), mups[:],
                    mybir.AluOpType.add, mybir.AluOpType.subtract,
                )
                d2 = work.tile([128, NSP], BF16, tag="d2")
                nc.vector.tensor_mul(d2[:], ctr[:], ctr[:])
                vrps = pp.tile([128, NSP], F32, tag="pp")
                nc.tensor.matmul(vrps[:], indssb[:], d2[:], start=True, stop=True)
                stdf = work.tile([128, NSP], BF16, tag="stdf")
                nc.scalar.activation(
                    stdf[:], vrps[:], mybir.ActivationFunctionType.Sqrt,
                    bias=col(6 * L + 2),
                )
                rstdf = work.tile([128, NSP], BF16, tag="rstdf")
                with nc.allow_low_precision(reason="groupnorm rstd in bf16"):
                    nc.vector.reciprocal(rstdf[:], stdf[:])
                hrT = work.tile([128, NSP], BF16, tag="hrT")
                nc.vector.tensor_mul(hrT[:], ctr[:], rstdf[:])

                # h update (gn affine + h_prime folded on host)
                h2ps = pp.tile([128, NSP], F32, tag="pp")
                nc.tensor.matmul(h2ps[:], w2a3[:, l, :], hrT[:], start=True, stop=True)
                hnT = work.tile([128, NSP], BF16, tag="hnT")
                nc.scalar.activation(
                    hnT[:], h2ps[:], mybir.ActivationFunctionType.Relu,
                    bias=col(6 * l + 4),
                )

                if l < 2:
                    # c-major gather + xbar-transpose rebuild of m-tiles
                    nc.sync.dma_start(g_in[l][:, :], hnT[:])
                    nc.gpsimd.collective_compute(
                        "AllGather", mybir.AluOpType.bypass,
                        replica_groups=RG,
                        ins=[g_in[l][:, :].opt()],
                        outs=[g_out[l][:, :, :].opt()],
                    )
                    for t in range(NT):
                        kk, j = t // 3, t % 3
                        eng = nc.sync if t % 2 == 0 else nc.scalar
                        eng.dma_start(
                            hnat[:, t * 128 : (t + 1) * 128],
                            g_out[l][kk, :, j * 128 : (j + 1) * 128],
                            transpose=True,
                        )
                    with nc.allow_low_precision(reason="fp8 h for diffusion"):
                        for t in range(NT):
                            if t % 2 == 0:
                                nc.vector.tensor_copy(
                                    hnf8[:, t * 128 : (t + 1) * 128],
                                    hnat[:, t * 128 : (t + 1) * 128],
                                )
                            else:
                                nc.scalar.copy(
                                    hnf8[:, t * 128 : (t + 1) * 128],
                                    hnat[:, t * 128 : (t + 1) * 128],
                                )
                else:
                    hmps = pp.tile([128, NSP], F32, tag="pp")
                    nc.tensor.matmul(hmps[:], ow1tsb[:], hnT[:], start=True, stop=True)
                    hmsb = work.tile([128, NSP], BF16, tag="hmsb")
                    nc.scalar.activation(
                        hmsb[:], hmps[:], mybir.ActivationFunctionType.Relu,
                        bias=col(6 * L),
                    )
                    oops = pp.tile([OUT, NSP], F32, tag="pp")
                    nc.tensor.matmul(oops[:], ow2tsb[:], hmsb[:], start=True, stop=True)
                    oosb = small.tile([OUT, NSP], F32R, tag="oosb")
                    nc.scalar.activation(
                        oosb[:], oops[:], mybir.ActivationFunctionType.Identity,
                        bias=colsb[0:OUT, 6 * L + 1 : 6 * L + 2],
                    )
                    nc.sync.dma_start(outt[:, :], oosb[:])

    nc.finalize()
    _NC_CACHE["nc"] = nc
    return nc


def _prep(inputs):
    f32 = np.float32

    def g(name):
        return np.asarray(inputs[name], f32)

    x, adj = g("x"), g("adj_list")
    alpha, transition = g("alpha"), g("transition")
    conv_w, conv_b = g("conv_w"), g("conv_b")
    w1, b1, eb1 = g("w1"), g("b1"), g("eb1")
    w2, b2, eb2 = g("w2"), g("b2"), g("eb2")
    gn_g, gn_b = g("gn_g"), g("gn_b")

    a = alpha - alpha.max(-1, keepdims=True)
    e = np.exp(a)
    srow = (e / e.sum(-1, keepdims=True)).sum(-1)          # [L,R]
    Wm = transition.mean(axis=2)                            # [L,R,C,C]
    Wp = (conv_w * srow)[:, :, None, None] * np.swapaxes(Wm, -1, -2)

    # h_prime path and groupnorm affine folded into the layer bias
    hp = np.zeros((C,), f32)
    b2eff = np.zeros((L, C), f32)
    for l in range(L):
        b2eff[l] = b2[l] + eb2[l] + w2[l][:, C:] @ hp + w2[l][:, :C] @ gn_b[l]
        hp = np.maximum(hp @ w1[l].T + b1[l] + eb1[l], 0.0).astype(f32)

    qkvo = np.stack(
        [np.swapaxes(g(w), -1, -2) for w in ("qw", "kw", "vw", "ow")], axis=1
    )  # [L,4,C,C] lhsT layout

    # w2a with gn_g folded: lhsT[c, o] = w2[l, o, c] * gn_g[l, c]
    w2at = np.swapaxes(w2[:, :, :C], -1, -2) * gn_g[:, :, None]  # [L,C,C]

    hid = np.arange(C) // HD
    same = (hid[:, None] == hid[None, :]).astype(f32)          # [C,C] same-head

    # centered ob: the part of ob surviving groupnorm mean subtraction
    ob = g("ob")
    obc = ob - ob @ (same / HD).T                               # [L,C]

    cols = np.zeros((C, NCOL), f32)
    for l in range(L):
        cols[:, 6 * l + 0] = g("qb")[l]
        cols[:, 6 * l + 1] = g("kb")[l]
        cols[:, 6 * l + 2] = g("vb")[l]
        cols[:, 6 * l + 3] = obc[l]
        cols[:, 6 * l + 4] = b2eff[l]
        cols[:, 6 * l + 5] = conv_b[l]
    cols[:, 6 * L] = g("out_b1")
    cols[:OUT, 6 * L + 1] = g("out_b2")
    cols[:, 6 * L + 2] = EPS

    bf = ml_dtypes.bfloat16
    consts = {
        "embt": np.concatenate([g("emb_w").T, g("emb_b")[None, :]], axis=0).astype(bf),
        "wp": np.ascontiguousarray(Wp.transpose(2, 0, 1, 3)).reshape(C, L * R * C).astype(bf),
        "qkvo": np.ascontiguousarray(qkvo.transpose(2, 0, 1, 3)).reshape(C, L * 4 * C).astype(bf),
        "w2at": np.ascontiguousarray(w2at.transpose(1, 0, 2)).reshape(C, L * C).astype(bf),
        "ow1t": np.ascontiguousarray(g("out_w1").T).astype(bf),
        "ow2t": np.ascontiguousarray(g("out_w2").T).astype(bf),
        "indb": same.astype(bf),
        "inds": (same / HD).astype(bf),
        "cols": cols,
    }

    xlast = x[:, :, -1, :]                                   # [B,N,DIN]
    fp8 = ml_dtypes.float8_e4m3
    in_maps = []
    for core in range(NCORES):
        b, s = core // 4, core % 4
        n0 = s * NS
        # adj block layout: A[m_pad, r, j] with m_pad = rank*384 + jj
        Ab = np.zeros((R, NSP, 4, NSP), f32)                 # [r, j, rank, jj]
        for s2 in range(4):
            Ab[:, :NS, s2, :NS] = adj[b][:, n0 : n0 + NS, s2 * NS : (s2 + 1) * NS]
        a3 = (
            Ab.transpose(2, 3, 0, 1)                          # [rank, jj, r, j]
            .reshape(MPAD, R, NSP)
            .reshape(NT, 128, R, NSP)
            .transpose(1, 0, 2, 3)                            # [mi, mt, r, j]
        )
        xtc = np.zeros((DIN + 1, MPAD), f32)
        xtc[DIN, :] = 1.0
        xv = xlast[b].T                                       # [DIN, N]
        for s2 in range(4):
            xtc[:DIN, s2 * NSP : s2 * NSP + NS] = xv[:, s2 * NS : (s2 + 1) * NS]
        in_maps.append(
            dict(
                consts,
                adjt=np.ascontiguousarray(a3).reshape(128, NT * R * NSP).astype(fp8),
                xt=xtc.astype(bf),
            )
        )
    return in_maps


def kernel(**inputs):
    nc = _build_nc()
    in_maps = _prep(inputs)
    res = run_bass_kernel_spmd(nc, in_maps, core_ids=list(range(NCORES)))
    out = np.zeros((B, N, OUT), np.float32)
    for core in range(NCORES):
        b, s = core // 4, core % 4
        out[b, s * NS : (s + 1) * NS, :] = np.asarray(
            res.results[core]["outt"], np.float32
        )[:, :NS].T
    return out


# revision 11
# speedup vs baseline: 1.3673x; 1.1694x over previous
"""MGDPR (gnn_message_passing) Trainium2 kernel, 8 NeuronCores.

Sharding: nodes row-sharded 4-way within each batch element; cores 0-3 own
batch 0, cores 4-7 own batch 1 (375 nodes each, padded to 384). The source
(m) axis uses a padded block order: rank k's nodes sit at m = k*384 + j, so
the AllGather output maps 1:1 onto SBUF m-tiles. adj is host-cast to fp8e4
and pre-laid in the exact SBUF layout ([mi][mt, r, j]) so the load is a few
large fully-contiguous DMAs. All per-node tensors are channel-major on chip
([C, nodes]); h is kept bf16. The embedding is fused into layer-0 diffusion
mt-major so compute rides the adjacency DMA stream. Between layers, h is
gathered c-major over the 4 cores of each batch (bf16, one AllGather), and
m-tiles are rebuilt with 12 DMA-transposes (xbar). GroupNorm's affine
(gn_g/gn_b) is folded into w2/bias on the host; h_prime's contribution
(zeros init + affine) folds into a per-layer bias.
"""

import numpy as np

try:
    import concourse.bass as bass
except ImportError:
    import sys

    sys.path.insert(0, "/opt/trn_rl_repo")
    import concourse.bass as bass

import ml_dtypes
import concourse.mybir as mybir
import concourse.tile as tile
from concourse import bacc
from concourse.bass_utils import run_bass_kernel_spmd

B, N, T, DIN, C, R, K, L, H, OUT = 2, 1500, 20, 32, 128, 5, 5, 3, 4, 2
HD = C // H
EPS = 1e-5
NCORES = 8
NS = 375            # real nodes per core
NSP = 384           # padded nodes per core (3 * 128)
NT = 12             # m tiles: 4 ranks * 3 tiles
MPAD = NT * 128     # 1536 = 4 * NSP
RG = [[0, 1, 2, 3], [4, 5, 6, 7]]
F32 = mybir.dt.float32
F32R = mybir.dt.float32r
BF16 = mybir.dt.bfloat16
FP8 = mybir.dt.float8e4
NCOL = 6 * L + 3    # bias columns

_NC_CACHE = {}


def _build_nc():
    if "nc" in _NC_CACHE:
        return _NC_CACHE["nc"]
    nc = bacc.Bacc(None, target_bir_lowering=False, debug=False, num_devices=NCORES)

    adjt = nc.dram_tensor("adjt", [128, NT * R * NSP], FP8, kind="ExternalInput")
    xt = nc.dram_tensor("xt", [DIN + 1, MPAD], BF16, kind="ExternalInput")
    embt_d = nc.dram_tensor("embt", [DIN + 1, C], BF16, kind="ExternalInput")
    wp_d = nc.dram_tensor("wp", [C, L * R * C], BF16, kind="ExternalInput")
    qkvo_d = nc.dram_tensor("qkvo", [C, L * 4 * C], BF16, kind="ExternalInput")
    w2at_d = nc.dram_tensor("w2at", [C, L * C], BF16, kind="ExternalInput")
    ow1t_d = nc.dram_tensor("ow1t", [C, C], BF16, kind="ExternalInput")
    ow2t_d = nc.dram_tensor("ow2t", [C, OUT], BF16, kind="ExternalInput")
    indb_d = nc.dram_tensor("indb", [C, C], BF16, kind="ExternalInput")
    inds_d = nc.dram_tensor("inds", [C, C], BF16, kind="ExternalInput")
    ident_d = nc.dram_tensor("ident", [C, C], BF16, kind="ExternalInput")
    cols_d = nc.dram_tensor("cols", [C, NCOL], F32, kind="ExternalInput")

    outt = nc.dram_tensor("outt", [OUT, NSP], F32R, kind="ExternalOutput")

    g_in = [nc.dram_tensor(f"g_in_{l}", [C, NSP], BF16) for l in range(2)]
    g_out = [nc.dram_tensor(f"g_out_{l}", [4, C, NSP], BF16) for l in range(2)]

    with tile.TileContext(nc) as tc:
        with (
            tc.tile_pool(name="persist", bufs=1) as pers,
            tc.tile_pool(name="work", bufs=2) as work,
            tc.tile_pool(name="zwork", bufs=2) as zwork,
            tc.tile_pool(name="small", bufs=2) as small,
            tc.tile_pool(name="pz", bufs=5, space="PSUM") as pz,
            tc.tile_pool(name="pp", bufs=2, space="PSUM") as pp,
            tc.tile_pool(name="pm", bufs=1, space="PSUM") as pm,
        ):
            # ---------- resident tensors ----------
            adjsb = pers.tile([128, NT, R, NSP], FP8, tag="adjsb")
            hnf8 = pers.tile([128, NT * 128], FP8, tag="hnf8")
            xtsb = pers.tile([DIN + 1, MPAD], BF16, tag="xtsb")
            embtsb = pers.tile([DIN + 1, C], BF16, tag="embtsb")
            wpsb = pers.tile([C, L * R * C], BF16, tag="wpsb")
            qkvosb = pers.tile([C, L * 4 * C], BF16, tag="qkvosb")
            w2atsb = pers.tile([C, L * C], BF16, tag="w2atsb")
            ow1tsb = pers.tile([C, C], BF16, tag="ow1tsb")
            ow2tsb = pers.tile([C, OUT], BF16, tag="ow2tsb")
            indbsb = pers.tile([C, C], BF16, tag="indbsb")
            indssb = pers.tile([C, C], BF16, tag="indssb")
            identsb = pers.tile([C, C], BF16, tag="identsb")
            colsb = pers.tile([C, NCOL], F32, tag="colsb")

            # small consts first (scalar queue), then xt, then adj stream
            for dst, src in (
                (embtsb, embt_d), (indbsb, indb_d), (indssb, inds_d),
                (identsb, ident_d),
                (colsb, cols_d), (qkvosb, qkvo_d),
                (wpsb, wp_d), (w2atsb, w2at_d), (ow1tsb, ow1t_d),
                (ow2tsb, ow2t_d),
            ):
                nc.scalar.dma_start(dst[:], src[:, :])
            nc.sync.dma_start(xtsb[:], xt[:, :])
            adjflat = adjsb.rearrange("p a b c -> p (a b c)")
            CH = 2 * R * NSP
            for ch in range(NT // 2):
                eng = nc.sync if ch % 2 == 0 else nc.gpsimd
                eng.dma_start(
                    adjflat[:, ch * CH : (ch + 1) * CH],
                    adjt[:, ch * CH : (ch + 1) * CH],
                )

            def col(i):
                return colsb[:, i : i + 1]

            wp4 = wpsb.rearrange("p (l r co) -> p l r co", l=L, r=R)
            qk4 = qkvosb.rearrange("p (l i co) -> p l i co", l=L, i=4)
            w2a3 = w2atsb.rearrange("p (l co) -> p l co", l=L)

            def embed(mt):
                ep = pp.tile([128, 128], F32, tag="pp")
                nc.tensor.matmul(
                    ep[:], xtsb[:, mt * 128 : (mt + 1) * 128], embtsb[:],
                    start=True, stop=True, skip_group_check=True,
                )
                with nc.allow_low_precision(reason="fp8 h for diffusion"):
                    if mt % 2 == 0:
                        nc.vector.tensor_copy(
                            hnf8[:, mt * 128 : (mt + 1) * 128], ep[:]
                        )
                    else:
                        nc.scalar.copy(hnf8[:, mt * 128 : (mt + 1) * 128], ep[:])

            for l in range(L):
                # ---- diffusion, mt-major with 5 accumulators ----
                if l == 0:
                    embed(0)
                    embed(1)
                zps = [
                    pz.tile([128, NSP], F32, tag="z", name=f"zp_{l}_{r}")
                    for r in range(R)
                ]
                h3 = hnf8.rearrange("p (mt c) -> p mt c", mt=NT)
                for m2 in range(NT // 2):
                    if l == 0 and 2 * m2 + 3 < NT:
                        embed(2 * m2 + 2)
                        embed(2 * m2 + 3)
                    for r in range(R):
                        nc.tensor.matmul(
                            zps[r][:],
                            h3[:, 2 * m2 : 2 * m2 + 2, :],
                            adjsb[:, 2 * m2 : 2 * m2 + 2, r, :],
                            start=(m2 == 0), stop=(m2 == NT // 2 - 1),
                            perf_mode=mybir.MatmulPerfMode.DoubleRow,
                            skip_group_check=True,
                        )
                mps = pm.tile([128, NSP], F32, tag="m")
                for r in range(R):
                    zsb = zwork.tile([128, NSP], BF16, tag="zsb")
                    if r % 2 == 0:
                        nc.scalar.copy(zsb[:], zps[r][:])
                    else:
                        nc.vector.tensor_copy(zsb[:], zps[r][:])
                    nc.tensor.matmul(
                        mps[:], wp4[:, l, r, :], zsb[:],
                        start=(r == 0), stop=(r == R - 1),
                        skip_group_check=True,
                    )
                hdT = work.tile([128, NSP], BF16, tag="hdT")
                nc.scalar.activation(
                    hdT[:], mps[:], mybir.ActivationFunctionType.Relu,
                    bias=col(6 * l + 5), scale=1.0,
                )

                # ---- retention (S=1), channel-major, block matmuls ----
                qps = pp.tile([128, NSP], F32, tag="pp")
                nc.tensor.matmul(qps[:], qk4[:, l, 0, :], hdT[:], start=True, stop=True)
                kps = pp.tile([128, NSP], F32, tag="pp")
                nc.tensor.matmul(kps[:], qk4[:, l, 1, :], hdT[:], start=True, stop=True)
                qsb = work.tile([128, NSP], BF16, tag="qsb")
                nc.scalar.activation(
                    qsb[:], qps[:], mybir.ActivationFunctionType.Identity,
                    bias=col(6 * l + 0),
                )
                ksb = work.tile([128, NSP], BF16, tag="ksb")
                nc.vector.tensor_scalar_add(ksb[:], kps[:], col(6 * l + 1))
                qk = work.tile([128, NSP], BF16, tag="qk")
                nc.vector.tensor_mul(qk[:], qsb[:], ksb[:])
                sbps = pp.tile([128, NSP], F32, tag="pp")
                nc.tensor.matmul(sbps[:], indbsb[:], qk[:], start=True, stop=True)
                vps = pp.tile([128, NSP], F32, tag="pp")
                nc.tensor.matmul(vps[:], qk4[:, l, 2, :], hdT[:], start=True, stop=True)
                vsb = work.tile([128, NSP], BF16, tag="vsb")
                nc.scalar.activation(
                    vsb[:], vps[:], mybir.ActivationFunctionType.Identity,
                    bias=col(6 * l + 2),
                )
                osb = work.tile([128, NSP], BF16, tag="osb")
                nc.vector.tensor_mul(osb[:], vsb[:], sbps[:])

                o2ps = pp.tile([128, NSP], F32, tag="pp")
                nc.tensor.matmul(o2ps[:], qk4[:, l, 3, :], osb[:], start=True, stop=True)
                o2sb = work.tile([128, NSP], BF16, tag="o2sb")
                nc.vector.tensor_copy(o2sb[:], o2ps[:])
                # centered o2 (+ centered ob folded on host): ctr = o2 + obc - mu
                mups = pp.tile([128, NSP], F32, tag="pp")
                nc.tensor.matmul(mups[:], indssb[:], o2sb[:], start=True, stop=True)
                ctr = work.tile([128, NSP], BF16, tag="ctr")
                nc.vector.scalar_tensor_tensor(
                    ctr[:], o2sb[:], col(6 * l + 3), mups[:],
                    mybir.AluOpType.add, mybir.AluOpType.subtract,
                )
                d2 = work.tile([128, NSP], BF16, tag="d2")
                nc.vector.tensor_mul(d2[:], ctr[:], ctr[:])
                vrps = pp.tile([128, NSP], F32, tag="pp")
                nc.tensor.matmul(vrps[:], indssb[:], d2[:], start=True, stop=True)
                stdf = work.tile([128, NSP], BF16, tag="stdf")
                nc.scalar.activation(
                    stdf[:], vrps[:], mybir.ActivationFunctionType.Sqrt,
                    bias=col(6 * L + 2),
                )
                rstdf = work.tile([128, NSP], BF16, tag="rstdf")
                with nc.allow_low_precision(reason="groupnorm rstd in bf16"):
                    nc.vector.reciprocal(rstdf[:], stdf[:])
                hrT = work.tile([128, NSP], BF16, tag="hrT")
                nc.vector.tensor_mul(hrT[:], ctr[:], rstdf[:])

                # h update (gn affine + h_prime folded on host)
                h2ps = pp.tile([128, NSP], F32, tag="pp")
                nc.tensor.matmul(h2ps[:], w2a3[:, l, :], hrT[:], start=True, stop=True)
                hnT = work.tile([128, NSP], BF16, tag="hnT")
                nc.scalar.activation(
                    hnT[:], h2ps[:], mybir.ActivationFunctionType.Relu,
                    bias=col(6 * l + 4),
                )

                if l < 2:
                    # c-major gather + xbar-transpose rebuild of m-tiles
                    nc.sync.dma_start(g_in[l][:, :], hnT[:])
                    nc.gpsimd.collective_compute(
                        "AllGather", mybir.AluOpType.bypass,
                        replica_groups=RG,
                        ins=[g_in[l][:, :].opt()],
                        outs=[g_out[l][:, :, :].opt()],
                    )
                    gsb = work.tile([128, 4 * NSP], BF16, tag="gsb")
                    for kk in range(4):
                        eng = nc.sync if kk % 2 == 0 else nc.scalar
                        eng.dma_start(
                            gsb[:, kk * NSP : (kk + 1) * NSP], g_out[l][kk, :, :]
                        )
                    with nc.allow_low_precision(reason="fp8 h for diffusion"):
                        for t in range(NT):
                            trp = pp.tile([128, 128], BF16, tag="pp", name=f"trp_{l}_{t}")
                            nc.tensor.transpose(
                                trp[:], gsb[:, t * 128 : (t + 1) * 128], identsb[:]
                            )
                            if t % 2 == 0:
                                nc.vector.tensor_copy(
                                    hnf8[:, t * 128 : (t + 1) * 128], trp[:]
                                )
                            else:
                                nc.scalar.copy(
                                    hnf8[:, t * 128 : (t + 1) * 128], trp[:]
                                )
                else:
                    hmps = pp.tile([128, NSP], F32, tag="pp")
                    nc.tensor.matmul(hmps[:], ow1tsb[:], hnT[:], start=True, stop=True)
                    hmsb = work.tile([128, NSP], BF16, tag="hmsb")
                    nc.scalar.activation(
                        hmsb[:], hmps[:], mybir.ActivationFunctionType.Relu,
                        bias=col(6 * L),
                    )
                    oops = pp.tile([OUT, NSP], F32, tag="pp")
                    nc.tensor.matmul(oops[:], ow2tsb[:], hmsb[:], start=True, stop=True)
                    oosb = small.tile([OUT, NSP], F32R, tag="oosb")
                    nc.scalar.activation(
                        oosb[:], oops[:], mybir.ActivationFunctionType.Identity,
                        bias=colsb[0:OUT, 6 * L + 1 : 6 * L + 2],
                    )
                    nc.sync.dma_start(outt[:, :], oosb[:])

    nc.finalize()
    _NC_CACHE["nc"] = nc
    return nc


def _prep(inputs):
    f32 = np.float32

    def g(name):
        return np.asarray(inputs[name], f32)

    x, adj = g("x"), g("adj_list")
    alpha, transition = g("alpha"), g("transition")
    conv_w, conv_b = g("conv_w"), g("conv_b")
    w1, b1, eb1 = g("w1"), g("b1"), g("eb1")
    w2, b2, eb2 = g("w2"), g("b2"), g("eb2")
    gn_g, gn_b = g("gn_g"), g("gn_b")

    a = alpha - alpha.max(-1, keepdims=True)
    e = np.exp(a)
    srow = (e / e.sum(-1, keepdims=True)).sum(-1)          # [L,R]
    Wm = transition.mean(axis=2)                            # [L,R,C,C]
    Wp = (conv_w * srow)[:, :, None, None] * np.swapaxes(Wm, -1, -2)

    # h_prime path and groupnorm affine folded into the layer bias
    hp = np.zeros((C,), f32)
    b2eff = np.zeros((L, C), f32)
    for l in range(L):
        b2eff[l] = b2[l] + eb2[l] + w2[l][:, C:] @ hp + w2[l][:, :C] @ gn_b[l]
        hp = np.maximum(hp @ w1[l].T + b1[l] + eb1[l], 0.0).astype(f32)

    qkvo = np.stack(
        [np.swapaxes(g(w), -1, -2) for w in ("qw", "kw", "vw", "ow")], axis=1
    )  # [L,4,C,C] lhsT layout

    # w2a with gn_g folded: lhsT[c, o] = w2[l, o, c] * gn_g[l, c]
    w2at = np.swapaxes(w2[:, :, :C], -1, -2) * gn_g[:, :, None]  # [L,C,C]

    hid = np.arange(C) // HD
    same = (hid[:, None] == hid[None, :]).astype(f32)          # [C,C] same-head

    # centered ob: the part of ob surviving groupnorm mean subtraction
    ob = g("ob")
    obc = ob - ob @ (same / HD).T                               # [L,C]

    cols = np.zeros((C, NCOL), f32)
    for l in range(L):
        cols[:, 6 * l + 0] = g("qb")[l]
        cols[:, 6 * l + 1] = g("kb")[l]
        cols[:, 6 * l + 2] = g("vb")[l]
        cols[:, 6 * l + 3] = obc[l]
        cols[:, 6 * l + 4] = b2eff[l]
        cols[:, 6 * l + 5] = conv_b[l]
    cols[:, 6 * L] = g("out_b1")
    cols[:OUT, 6 * L + 1] = g("out_b2")
    cols[:, 6 * L + 2] = EPS

    bf = ml_dtypes.bfloat16
    consts = {
        "embt": np.concatenate([g("emb_w").T, g("emb_b")[None, :]], axis=0).astype(bf),
        "wp": np.ascontiguousarray(Wp.transpose(2, 0, 1, 3)).reshape(C, L * R * C).astype(bf),
        "qkvo": np.ascontiguousarray(qkvo.transpose(2, 0, 1, 3)).reshape(C, L * 4 * C).astype(bf),
        "w2at": np.ascontiguousarray(w2at.transpose(1, 0, 2)).reshape(C, L * C).astype(bf),
        "ow1t": np.ascontiguousarray(g("out_w1").T).astype(bf),
        "ow2t": np.ascontiguousarray(g("out_w2").T).astype(bf),
        "indb": same.astype(bf),
        "inds": (same / HD).astype(bf),
        "ident": np.eye(C, dtype=f32).astype(bf),
        "cols": cols,
    }

    xlast = x[:, :, -1, :]                                   # [B,N,DIN]
    fp8 = ml_dtypes.float8_e4m3
    in_maps = []
    for core in range(NCORES):
        b, s = core // 4, core % 4
        n0 = s * NS
        # adj block layout: A[m_pad, r, j] with m_pad = rank*384 + jj
        Ab = np.zeros((R, NSP, 4, NSP), f32)                 # [r, j, rank, jj]
        for s2 in range(4):
            Ab[:, :NS, s2, :NS] = adj[b][:, n0 : n0 + NS, s2 * NS : (s2 + 1) * NS]
        a3 = (
            Ab.transpose(2, 3, 0, 1)                          # [rank, jj, r, j]
            .reshape(MPAD, R, NSP)
            .reshape(NT, 128, R, NSP)
            .transpose(1, 0, 2, 3)                            # [mi, mt, r, j]
        )
        xtc = np.zeros((DIN + 1, MPAD), f32)
        xtc[DIN, :] = 1.0
        xv = xlast[b].T                                       # [DIN, N]
        for s2 in range(4):
            xtc[:DIN, s2 * NSP : s2 * NSP + NS] = xv[:, s2 * NS : (s2 + 1) * NS]
        in_maps.append(
            dict(
                consts,
                adjt=np.ascontiguousarray(a3).reshape(128, NT * R * NSP).astype(fp8),
                xt=xtc.astype(bf),
            )
        )
    return in_maps


def kernel(**inputs):
    nc = _build_nc()
    in_maps = _prep(inputs)
    res = run_bass_kernel_spmd(nc, in_maps, core_ids=list(range(NCORES)))
    out = np.zeros((B, N, OUT), np.float32)
    for core in range(NCORES):
        b, s = core // 4, core % 4
        out[b, s * NS : (s + 1) * NS, :] = np.asarray(
            res.results[core]["outt"], np.float32
        )[:, :NS].T
    return out
